# revision 1
# baseline (speedup 1.0000x reference)
"""CG solve of (S + 500 I) Z = S X^T with S = X_coo^T X_coo, distributed
over 8 TRN2 NeuronCores.

Strategy:
  - Host: materialize S (16384x16384 f32) from the COO arrays (scipy), fold
    the +lambda*I into it, split into bf16 hi/lo pair (hi+lo ~ 18-bit
    mantissa), and 1D-partition the columns across the 8 cores
    (16384 x 2048 per core).
  - Device (SPMD x8): CG on the full batch of 64 RHS. Each core computes its
    2048-item slice of each matvec as 3 accumulated bf16 matmuls
    (hi*hi + hi*lo + lo*hi) streaming its S slice from HBM (memory-bound),
    then an AllGather assembles the full matvec result on every core.
    CG state is replicated; vector updates are per-partition-scalar fused
    DVE ops in a (batch x half) layout; the matvec input is re-transposed
    to items-major via TensorE each iteration.
  - 10 CG iterations (residual reaches the f32 floor by ~iter 8; the
    reference's early-out freeze triggers there too, so both are the
    converged solution).
"""
import sys
import types

import numpy as np

N_CORES = 8
N_ITEMS = 16384
BATCH = 64
HALF = N_ITEMS // 2          # 8192
SLICE = N_ITEMS // N_CORES   # 2048
LAM = np.float32(500.0)
N_ITERS = 5
N_SPLIT_ITERS = 2   # accurate (hi+lo) matvecs; later iterations run hi-only
                    # (inexact-Krylov relaxation: late matvecs tolerate error).
                    # Convergence hits the bf16-split floor (2.7e-4) at iter 4;
                    # the y matvec always runs split (RHS accuracy is critical).
                    # Schedule validated in numpy: 5 iters @ 2 split = 2.65e-4.
KTILES = 128                 # contraction tiles of 128 items
KT_PER_DMA = 2               # k-tiles per S-slab DMA

last_exec_time_ns = None


def _install_ntff_hook():
    if "antenv.axon_hooks" in sys.modules:
        return
    try:
        from trn_agent_boot.trn_boot import _ntff_profile_via_ctypes

        hook = _ntff_profile_via_ctypes("/opt/axon/libaxon_pjrt.so")
        mod = types.ModuleType("antenv.axon_hooks")
        mod.get_axon_ntff_profile_hook = lambda: hook
        mod.set_axon_ntff_profile_hook = lambda h: None
        sys.modules["antenv.axon_hooks"] = mod
    except Exception:
        pass


def _build_bass():
    import concourse.bass as bass  # noqa: F401
    import concourse.mybir as mybir
    import concourse.tile as tile
    from concourse import bacc
    from concourse.masks import make_identity

    F32 = mybir.dt.float32
    BF16 = mybir.dt.bfloat16
    ALU = mybir.AluOpType

    nc = bacc.Bacc(
        "TRN2",
        target_bir_lowering=False,
        debug=False,
        enable_asserts=False,
        num_devices=N_CORES,
    )

    # Inputs (per core)
    s_hi_in = nc.dram_tensor("s_hi", [N_ITEMS, SLICE], BF16, kind="ExternalInput").ap()
    s_lo_in = nc.dram_tensor("s_lo", [N_ITEMS, SLICE], BF16, kind="ExternalInput").ap()
    xt_hi_in = nc.dram_tensor("xt_hi", [128, HALF], BF16, kind="ExternalInput").ap()
    xt_lo_in = nc.dram_tensor("xt_lo", [128, HALF], BF16, kind="ExternalInput").ap()
    xst_in = nc.dram_tensor("xst", [128, HALF], F32, kind="ExternalInput").ap()
    z_out = nc.dram_tensor("z_out", [128, HALF], F32, kind="ExternalOutput").ap()

    # k-tile slab views of the S inputs: slab g covers items [128g, 128g+128)
    s_hi_t = s_hi_in.rearrange("(g ki) m -> g ki m", ki=128)
    s_lo_t = s_lo_in.rearrange("(g ki) m -> g ki m", ki=128)

    with tile.TileContext(nc) as tc:
        with (
            tc.tile_pool(name="state", bufs=1) as state_pool,
            tc.tile_pool(name="slab", bufs=2) as slab_pool,
            tc.tile_pool(name="wrk", bufs=1) as wrk_pool,
            tc.tile_pool(name="sc", bufs=1) as sc_pool,
            tc.tile_pool(name="ps", bufs=1, space="PSUM") as ps_pool,
            tc.tile_pool(name="tps", bufs=3, space="PSUM") as tps_pool,
            tc.tile_pool(name="dram", bufs=2, space="DRAM") as dram_pool,
        ):
            P_st = state_pool.tile([128, HALF], F32, name="P_st")
            R_st = state_pool.tile([128, HALF], F32, name="R_st")
            X_st = state_pool.tile([128, HALF], F32, name="X_st")
            A_st = state_pool.tile([128, HALF], F32, name="A_st")
            P_hi = state_pool.tile([128, HALF], BF16, name="P_hi")
            P_lo = state_pool.tile([128, HALF], BF16, name="P_lo")
            ident = sc_pool.tile([128, 128], F32, name="ident")
            make_identity(nc, ident[:])
            # 64x64 identity replicated on both partition halves (PE transpose
            # requires identity at the same base partition as the source).
            ident64 = sc_pool.tile([128, 64], F32, name="ident64")
            nc.vector.tensor_copy(ident64[0:64, :], ident[0:64, 0:64])
            nc.sync.dma_start(ident64[64:128, :], ident[0:64, 0:64])

            partials = sc_pool.tile([128, 4], F32, name="partials")
            rpartials = sc_pool.tile([128, 4], F32, name="rpartials")
            pap128 = sc_pool.tile([128, 1], F32, name="pap128")
            rsn128 = sc_pool.tile([128, 1], F32, name="rsn128")
            tmp64 = sc_pool.tile([64, 1], F32, name="tmp64")
            pap64 = sc_pool.tile([64, 1], F32, name="pap64")
            rsn64 = sc_pool.tile([64, 1], F32, name="rsn64")
            rs_old = sc_pool.tile([64, 1], F32, name="rs_old")
            inv64 = sc_pool.tile([64, 1], F32, name="inv64")
            alpha = sc_pool.tile([128, 1], F32, name="alpha")
            nalpha = sc_pool.tile([128, 1], F32, name="nalpha")
            beta = sc_pool.tile([128, 1], F32, name="beta")

            def matvec(lhs_hi, lhs_lo, split=True):
                """A_st <- (S' @ p) in state layout, via local slice + AllGather.
                lhs_hi/lhs_lo: (128, HALF) bf16 items-major lhsT tiles.
                split=False streams/computes only the bf16 hi product."""
                ag_in = dram_pool.tile([BATCH, SLICE], F32, name="ag_in", tag="ag_in")
                ag_out = dram_pool.tile(
                    [BATCH * N_CORES, SLICE], F32, name="ag_out",
                    addr_space="Shared", tag="ag_out",
                )
                psum = ps_pool.tile([BATCH, SLICE], F32, name="mv_psum")
                for gd in range(KTILES // KT_PER_DMA):
                    hi_slab = slab_pool.tile(
                        [128, KT_PER_DMA * SLICE], BF16, name="hi_slab"
                    )
                    hi_view = hi_slab[:].rearrange("ki (u m) -> ki u m", u=KT_PER_DMA)
                    nc.sync.dma_start(
                        hi_view,
                        s_hi_t[gd * KT_PER_DMA : (gd + 1) * KT_PER_DMA].transpose(
                            [1, 0, 2]
                        ),
                    )
                    if split:
                        lo_slab = slab_pool.tile(
                            [128, KT_PER_DMA * SLICE], BF16, name="lo_slab"
                        )
                        lo_view = lo_slab[:].rearrange(
                            "ki (u m) -> ki u m", u=KT_PER_DMA
                        )
                        nc.sync.dma_start(
                            lo_view,
                            s_lo_t[gd * KT_PER_DMA : (gd + 1) * KT_PER_DMA].transpose(
                                [1, 0, 2]
                            ),
                        )
                    for u in range(KT_PER_DMA):
                        g = gd * KT_PER_DMA + u
                        wh = lhs_hi[:, g * BATCH : (g + 1) * BATCH]
                        first = g == 0
                        last = g == KTILES - 1
                        for nt in range(SLICE // 512):
                            rh = hi_slab[:, u * SLICE + nt * 512 : u * SLICE + (nt + 1) * 512]
                            po = psum[:, nt * 512 : (nt + 1) * 512]
                            if split:
                                wl = lhs_lo[:, g * BATCH : (g + 1) * BATCH]
                                rl = lo_slab[:, u * SLICE + nt * 512 : u * SLICE + (nt + 1) * 512]
                                nc.tensor.matmul(po, lhsT=wh, rhs=rh, start=first, stop=False)
                                nc.tensor.matmul(po, lhsT=wh, rhs=rl, start=False, stop=False)
                                nc.tensor.matmul(po, lhsT=wl, rhs=rh, start=False, stop=last)
                            else:
                                nc.tensor.matmul(po, lhsT=wh, rhs=rh, start=first, stop=last)
                # psum (64, 2048) batch-major local slice -> SBUF -> AG
                a_loc = wrk_pool.tile([BATCH, SLICE], F32, name="a_loc", tag="w_dot")
                nc.vector.tensor_copy(a_loc[:], psum[:])
                nc.sync.dma_start(ag_in[:], a_loc[:])
                nc.gpsimd.collective_compute(
                    "AllGather",
                    ALU.bypass,
                    replica_groups=[list(range(N_CORES))],
                    ins=[ag_in[:].opt()],
                    outs=[ag_out[:].opt()],
                )
                # scatter the 8 rank blocks into state layout
                for r in range(N_CORES):
                    h, q = r // 4, r % 4
                    nc.sync.dma_start(
                        A_st[64 * h : 64 * h + 64, q * SLICE : (q + 1) * SLICE],
                        ag_out[64 * r : 64 * r + 64, :],
                    )

            def dot_state(a_t, b_t, out_parts, out128):
                """per-batch-partition dot partials: out128[p] = sum_j a*b."""
                for c in range(4):
                    w = wrk_pool.tile([128, SLICE], F32, name="w_dot")
                    sl = slice(c * SLICE, (c + 1) * SLICE)
                    nc.vector.tensor_tensor(
                        out=w[:], in0=a_t[:, sl], in1=b_t[:, sl], op=ALU.mult
                    )
                    nc.vector.reduce_sum(
                        out_parts[:, c : c + 1], w[:], axis=mybir.AxisListType.X
                    )
                nc.vector.reduce_sum(out128[:], out_parts[:], axis=mybir.AxisListType.X)

            def fold_half(in128, out64):
                """out64 = in128[0:64] + in128[64:128]"""
                nc.sync.dma_start(tmp64[:], in128[64:128, 0:1])
                nc.vector.tensor_tensor(
                    out=out64[:], in0=tmp64[:], in1=in128[0:64, 0:1], op=ALU.add
                )

            def transpose_split(src_st, dst_hi, dst_lo, need_lo=True):
                """src (128,HALF) f32 state layout -> items-major bf16 hi/lo.
                8 transpose blocks share one PSUM bank so the hi-cast and
                lo-subtract run as one 512-wide op each instead of 128 tiny
                per-block copies (ACT-bound otherwise)."""
                for h in range(2):
                    for jg in range(HALF // 128 // 8):
                        tp = tps_pool.tile([128, 512], F32, name="tp")
                        for k in range(8):
                            jc = jg * 8 + k
                            nc.tensor.transpose(
                                tp[:, k * 64 : (k + 1) * 64],
                                src_st[64 * h : 64 * h + 64, jc * 128 : (jc + 1) * 128],
                                ident64[64 * h : 64 * h + 64, :],
                            )
                        c0 = (h * 64 + jg * 8) * BATCH
                        hi_blk = dst_hi[:, c0 : c0 + 512]
                        nc.vector.tensor_copy(hi_blk, tp[:])
                        if need_lo:
                            nc.vector.tensor_tensor(
                                out=dst_lo[:, c0 : c0 + 512],
                                in0=tp[:],
                                in1=hi_blk,
                                op=ALU.subtract,
                            )

            # ---- y = S' x_t - lam x_t ; init CG state ----
            # xst_in holds (-lam * x) in state layout; stage it in X_st,
            # which is dead until iteration 0 overwrites it.
            nc.sync.dma_start(X_st[:], xst_in)
            nc.sync.dma_start(P_hi[:], xt_hi_in)
            nc.sync.dma_start(P_lo[:], xt_lo_in)
            matvec(P_hi[:], P_lo[:])
            # R = A + (-lam x) ; P = R
            nc.vector.tensor_tensor(out=R_st[:], in0=A_st[:], in1=X_st[:], op=ALU.add)
            nc.vector.tensor_copy(P_st[:], R_st[:])
            dot_state(R_st[:], R_st[:], rpartials, rsn128[:])
            fold_half(rsn128[:], rs_old[:])

            # ---- CG iterations ----
            for it in range(N_ITERS):
                split = it < N_SPLIT_ITERS
                transpose_split(P_st[:], P_hi[:], P_lo[:], need_lo=split)
                matvec(P_hi[:], P_lo[:], split=split)
                # pap = dot(P, A)
                dot_state(P_st[:], A_st[:], partials, pap128[:])
                fold_half(pap128[:], pap64[:])
                nc.vector.tensor_scalar_add(pap64[:], pap64[:], 1e-12)
                nc.vector.reciprocal(inv64[:], pap64[:])
                nc.vector.tensor_tensor(
                    out=alpha[0:64, 0:1], in0=rs_old[:], in1=inv64[:], op=ALU.mult
                )
                nc.sync.dma_start(alpha[64:128, 0:1], alpha[0:64, 0:1])
                nc.vector.tensor_scalar_mul(nalpha[:], alpha[:], -1.0)
                # X += alpha * P   (first iteration: X = alpha * P)
                if it == 0:
                    nc.vector.tensor_scalar_mul(X_st[:], P_st[:], alpha[:])
                else:
                    nc.vector.scalar_tensor_tensor(
                        out=X_st[:], in0=P_st[:], scalar=alpha[:], in1=X_st[:],
                        op0=ALU.mult, op1=ALU.add,
                    )
                if it == N_ITERS - 1:
                    break
                # R -= alpha * A
                nc.vector.scalar_tensor_tensor(
                    out=R_st[:], in0=A_st[:], scalar=nalpha[:], in1=R_st[:],
                    op0=ALU.mult, op1=ALU.add,
                )
                # rs_new = dot(R, R); beta = rs_new / rs_old; rs_old = rs_new
                dot_state(R_st[:], R_st[:], rpartials, rsn128[:])
                fold_half(rsn128[:], rsn64[:])
                nc.vector.tensor_scalar_add(rs_old[:], rs_old[:], 1e-12)
                nc.vector.reciprocal(inv64[:], rs_old[:])
                nc.vector.tensor_tensor(
                    out=beta[0:64, 0:1], in0=rsn64[:], in1=inv64[:], op=ALU.mult
                )
                nc.sync.dma_start(beta[64:128, 0:1], beta[0:64, 0:1])
                nc.vector.tensor_copy(rs_old[:], rsn64[:])
                # P = R + beta * P
                nc.vector.scalar_tensor_tensor(
                    out=P_st[:], in0=P_st[:], scalar=beta[:], in1=R_st[:],
                    op0=ALU.mult, op1=ALU.add,
                )

            nc.sync.dma_start(z_out, X_st[:])

    nc.compile()
    return nc


_NC_CACHE = None


def kernel(X_batch, rows, cols, values, num_users):
    global last_exec_time_ns, _NC_CACHE
    import ml_dtypes
    import scipy.sparse as sp

    X_batch = np.ascontiguousarray(np.asarray(X_batch, dtype=np.float32))
    rows = np.asarray(rows).astype(np.int64).ravel()
    cols = np.asarray(cols).astype(np.int64).ravel()
    values = np.asarray(values, dtype=np.float32).ravel()
    nu = int(np.asarray(num_users))

    # ---- host: S' = X^T X + lam I, bf16 split, column shards ----
    Xs = sp.coo_matrix((values, (rows, cols)), shape=(nu, N_ITEMS)).tocsr()
    S = (Xs.T @ Xs).toarray().astype(np.float32, copy=False)
    S[np.arange(N_ITEMS), np.arange(N_ITEMS)] += LAM
    S_hi = S.astype(ml_dtypes.bfloat16)
    S_lo = (S - S_hi.astype(np.float32)).astype(ml_dtypes.bfloat16)
    del S

    xt = X_batch.T.astype(np.float32)                     # (items, batch)
    xt_t = np.ascontiguousarray(
        xt.reshape(KTILES, 128, BATCH).transpose(1, 0, 2).reshape(128, HALF)
    )
    xt_hi = xt_t.astype(ml_dtypes.bfloat16)
    xt_lo = (xt_t - xt_hi.astype(np.float32)).astype(ml_dtypes.bfloat16)
    xst = np.ascontiguousarray(
        np.concatenate([X_batch[:, :HALF], X_batch[:, HALF:]], axis=0)
    ) * np.float32(-LAM)

    in_maps = []
    for c in range(N_CORES):
        sl = slice(c * SLICE, (c + 1) * SLICE)
        in_maps.append(
            {
                "s_hi": np.ascontiguousarray(S_hi[:, sl]),
                "s_lo": np.ascontiguousarray(S_lo[:, sl]),
                "xt_hi": xt_hi,
                "xt_lo": xt_lo,
                "xst": xst,
            }
        )

    _install_ntff_hook()
    from concourse import bass_utils
    from concourse.bass_interp import get_hw_module

    if _NC_CACHE is None:
        nc = _build_bass()
        nc.m = get_hw_module(nc.m)
        _NC_CACHE = nc
    nc = _NC_CACHE

    try:
        res = bass_utils.run_bass_kernel_spmd(
            nc, in_maps, core_ids=list(range(N_CORES)), trace=True
        )
    except Exception:
        res = bass_utils.run_bass_kernel_spmd(
            nc, in_maps, core_ids=list(range(N_CORES)), trace=False
        )
    last_exec_time_ns = res.exec_time_ns

    z_st = res.results[0]["z_out"]                        # (128, HALF)
    Z = np.concatenate([z_st[0:64, :], z_st[64:128, :]], axis=1)  # (64, items)
    return Z.astype(np.float32)



# revision 16
# speedup vs baseline: 2.9763x; 2.9763x over previous
"""CG solve of (S + 500 I) Z = S X^T with S = X_coo^T X_coo, distributed
over 8 TRN2 NeuronCores.

Design (v2):
  - Host: S = X^T X dense (f32), shipped twice: bf16 (for the RHS pass
    y = S x, accuracy-critical) and fp8-e4m3 scaled (for the CG iteration
    matvecs, half the HBM traffic). Column-sharded 8 ways (16384 x 2048
    per core).
  - Matvec: out = lhsT.T @ rhs with lhsT = v items-major bf16 (64-wide
    batch) and rhs = streamed S slab (fp8 or bf16). Mixed bf16 x fp8 PE
    matmul (HW-validated by probe).
  - Algorithm: Chronopoulos-Gear CG (single reduction point per
    iteration): gamma=(r,r) is computed lazily during the previous
    matvec, delta=(w,r) during the post-AllGather scatter; then
    alpha/beta and all vector updates (p,q,r) are per-rank-block chunk
    ops whose transposes feed the next matvec incrementally (MM emission
    interleaved per block so TensorE never waits for the full update
    sweep). x is accumulated in DRAM off the critical path.
  - Iteration AllGathers carry bf16 (validated); y's carries f32.
  - 3 CG iterations. Numpy mirror of the exact device arithmetic:
    maxrel 6.7e-3 vs the 2e-2 gate.
"""
import sys
import types

import numpy as np

N_CORES = 8
N_ITEMS = 16384
BATCH = 64
HALF = N_ITEMS // 2          # 8192
SLICE = N_ITEMS // N_CORES   # 2048
LAM = float(500.0)
K_ITERS = 3
KTILES = 128
NBLK = 8                     # rank blocks per gathered matvec

last_exec_time_ns = None


def _install_ntff_hook():
    if "antenv.axon_hooks" in sys.modules:
        return
    try:
        from trn_agent_boot.trn_boot import _ntff_profile_via_ctypes

        hook = _ntff_profile_via_ctypes("/opt/axon/libaxon_pjrt.so")
        mod = types.ModuleType("antenv.axon_hooks")
        mod.get_axon_ntff_profile_hook = lambda: hook
        mod.set_axon_ntff_profile_hook = lambda h: None
        sys.modules["antenv.axon_hooks"] = mod
    except Exception:
        pass


def _build_bass(inv8: float):
    import concourse.bass as bass  # noqa: F401
    import concourse.mybir as mybir
    import concourse.tile as tile
    from concourse import bacc
    from concourse.masks import make_identity

    F32 = mybir.dt.float32
    BF16 = mybir.dt.bfloat16
    FP8 = mybir.dt.float8e4
    ALU = mybir.AluOpType
    ACT_COPY = mybir.ActivationFunctionType.Copy
    AXX = mybir.AxisListType.X

    nc = bacc.Bacc(
        "TRN2",
        target_bir_lowering=False,
        debug=False,
        enable_asserts=False,
        num_devices=N_CORES,
    )

    s8_in = nc.dram_tensor("s8", [N_ITEMS, SLICE], FP8, kind="ExternalInput").ap()
    shi_in = nc.dram_tensor("shi", [N_ITEMS, SLICE], BF16, kind="ExternalInput").ap()
    xh_in = nc.dram_tensor("xh", [128, HALF], BF16, kind="ExternalInput").ap()
    xl_in = nc.dram_tensor("xl", [128, HALF], BF16, kind="ExternalInput").ap()
    z_out = nc.dram_tensor("z_out", [128, HALF], F32, kind="ExternalOutput").ap()

    s8_t = s8_in.rearrange("(g ki) m -> g ki m", ki=128)
    shi_t = shi_in.rearrange("(g ki) m -> g ki m", ki=128)

    with tile.TileContext(nc) as tc:
        with (
            tc.tile_pool(name="state", bufs=1) as state_pool,
            tc.tile_pool(name="scr", bufs=3) as scr_pool,
            tc.tile_pool(name="slab", bufs=3) as slab_pool,
            tc.tile_pool(name="sc", bufs=1) as sc_pool,
            tc.tile_pool(name="ps", bufs=1, space="PSUM") as ps_pool,
            tc.tile_pool(name="tps", bufs=2, space="PSUM") as tps_pool,
            tc.tile_pool(name="dram", bufs=2, space="DRAM") as dram_pool,
            tc.tile_pool(name="dramx", bufs=1, space="DRAM") as dramx_pool,
        ):
            R_st = state_pool.tile([128, HALF], F32, name="R_st")
            Q_st = state_pool.tile([128, HALF], BF16, name="Q_st")
            P_st = state_pool.tile([128, HALF], BF16, name="P_st")
            T16 = state_pool.tile([128, HALF], BF16, name="T16")
            V_it = state_pool.tile([128, HALF], BF16, name="V_it")
            x_dram = dramx_pool.tile([128, HALF], F32, name="x_dram")

            ident = sc_pool.tile([128, 128], F32, name="ident")
            make_identity(nc, ident[:])
            ident64 = sc_pool.tile([128, 64], F32, name="ident64")
            nc.vector.tensor_copy(ident64[0:64, :], ident[0:64, 0:64])
            nc.sync.dma_start(ident64[64:128, :], ident[0:64, 0:64])

            gpart = sc_pool.tile([128, 4], F32, name="gpart")
            dpart = sc_pool.tile([128, 4], F32, name="dpart")
            g128 = sc_pool.tile([128, 1], F32, name="g128")
            d128 = sc_pool.tile([128, 1], F32, name="d128")
            tmp64 = sc_pool.tile([64, 1], F32, name="tmp64")
            gamma = sc_pool.tile([64, 1], F32, name="gamma")
            g_old = sc_pool.tile([64, 1], F32, name="g_old")
            delta = sc_pool.tile([64, 1], F32, name="delta")
            t1 = sc_pool.tile([64, 1], F32, name="t1")
            d2 = sc_pool.tile([64, 1], F32, name="d2")
            inv_s = sc_pool.tile([64, 1], F32, name="inv_s")
            inv_a_old = sc_pool.tile([64, 1], F32, name="inv_a_old")
            alpha64 = sc_pool.tile([64, 1], F32, name="alpha64")
            beta64 = sc_pool.tile([64, 1], F32, name="beta64")
            alpha128 = sc_pool.tile([128, 1], F32, name="alpha128")
            nalpha128 = sc_pool.tile([128, 1], F32, name="nalpha128")
            beta128 = sc_pool.tile([128, 1], F32, name="beta128")

            def blk(tile_ap, j):
                h, qq = j // 4, j % 4
                return tile_ap[64 * h : 64 * h + 64, qq * SLICE : (qq + 1) * SLICE]

            def half(tile_ap, j):
                h = j // 4
                return tile_ap[64 * h : 64 * h + 64, :]

            def part_col(parts, j):
                h = j // 4
                return parts[64 * h : 64 * h + 64, (j % 4) : (j % 4) + 1]

            def sca(vec128, j):
                h = j // 4
                return vec128[64 * h : 64 * h + 64, 0:1]

            mv_ps = [None]

            def emit_y_mm(gd):
                """y-pass matmuls for k-tile pair (2gd, 2gd+1)."""
                slab = slab_pool.tile([128, 2 * SLICE], BF16, name="yslab", tag="slab")
                view = slab[:].rearrange("ki (u m) -> ki u m", u=2)
                nc.sync.dma_start(
                    view, shi_t[2 * gd : 2 * gd + 2].transpose([1, 0, 2])
                )
                ps = mv_ps[0]
                for u in range(2):
                    g = 2 * gd + u
                    for nt in range(SLICE // 512):
                        rh = slab[:, u * SLICE + nt * 512 : u * SLICE + (nt + 1) * 512]
                        po = ps[:, nt * 512 : (nt + 1) * 512]
                        nc.tensor.matmul(
                            po, lhsT=xh_ref[0][:, g * 64 : (g + 1) * 64], rhs=rh,
                            start=(g == 0), stop=False,
                        )
                        nc.tensor.matmul(
                            po, lhsT=xh_ref[1][:, g * 64 : (g + 1) * 64], rhs=rh,
                            start=False, stop=(g == KTILES - 1),
                        )

            def emit_iter_mm(q):
                """iteration matvec matmuls for k-tile quad [4q, 4q+4)."""
                slab = slab_pool.tile([128, 4 * SLICE], FP8, name="fslab", tag="slab")
                view = slab[:].rearrange("ki (u m) -> ki u m", u=4)
                nc.sync.dma_start(
                    view, s8_t[4 * q : 4 * q + 4].transpose([1, 0, 2])
                )
                ps = mv_ps[0]
                for u in range(4):
                    g = 4 * q + u
                    for nt in range(SLICE // 512):
                        rh = slab[:, u * SLICE + nt * 512 : u * SLICE + (nt + 1) * 512]
                        nc.tensor.matmul(
                            ps[:, nt * 512 : (nt + 1) * 512],
                            lhsT=V_it[:, g * 64 : (g + 1) * 64], rhs=rh,
                            start=(g == 0), stop=(g == KTILES - 1),
                        )

            def finish_matvec(scale_inv8):
                """psum -> a_loc -> ag_in -> AllGather. Returns ag_out.

                The y pass (scale_inv8=False) gathers f32; iteration passes
                gather bf16 (halves the collective, validated numerically)
                with the fp8 descale folded into the copy.
                """
                ps = mv_ps[0]
                if scale_inv8:
                    ag_in = dram_pool.tile(
                        [BATCH, SLICE], BF16, name="ag16_in", tag="ag16_in"
                    )
                    ag_out = dram_pool.tile(
                        [BATCH * N_CORES, SLICE], BF16, name="ag16_out",
                        addr_space="Shared", tag="ag16_out",
                    )
                    a_loc = scr_pool.tile(
                        [128, SLICE], BF16, name="a_loc16", tag="scr16b"
                    )
                    nc.vector.tensor_scalar_mul(a_loc[0:64, :], ps[:], float(inv8))
                else:
                    ag_in = dram_pool.tile(
                        [BATCH, SLICE], F32, name="ag_in", tag="ag_in"
                    )
                    ag_out = dram_pool.tile(
                        [BATCH * N_CORES, SLICE], F32, name="ag_out",
                        addr_space="Shared", tag="ag_out",
                    )
                    a_loc = scr_pool.tile([128, SLICE], F32, name="a_loc", tag="scrx")
                    nc.vector.tensor_copy(a_loc[0:64, :], ps[:])
                nc.sync.dma_start(ag_in[:], a_loc[0:64, :])
                nc.gpsimd.collective_compute(
                    "AllGather",
                    ALU.bypass,
                    replica_groups=[list(range(N_CORES))],
                    ins=[ag_in[:].opt()],
                    outs=[ag_out[:].opt()],
                )
                return ag_out

            def transpose_block(j, src):
                """src block j (64, 2048) -> V_it items-major bf16."""
                h = j // 4
                cb = (j % 4) * SLICE
                for t8 in range(2):
                    tp = tps_pool.tile([128, 512], F32, name="tp")
                    for t in range(8):
                        tt = 8 * t8 + t
                        nc.tensor.transpose(
                            tp[:, t * 64 : (t + 1) * 64],
                            src[64 * h : 64 * h + 64, cb + 128 * tt : cb + 128 * (tt + 1)],
                            ident64[64 * h : 64 * h + 64, :],
                        )
                    c0 = (16 * j + 8 * t8) * 64
                    nc.scalar.activation(V_it[:, c0 : c0 + 512], tp[:], ACT_COPY)

            # ================= phase 0: load weights, y matvec =================
            mv_ps[0] = ps_pool.tile([BATCH, SLICE], F32, name="mv_ps")
            xh_ref = [None, None]
            with tc.tile_pool(name="yw", bufs=1) as yw_pool:
                xh_ref[0] = yw_pool.tile([128, HALF], BF16, name="xh_t")
                xh_ref[1] = yw_pool.tile([128, HALF], BF16, name="xl_t")
                nc.sync.dma_start(xh_ref[0][:], xh_in)
                nc.sync.dma_start(xh_ref[1][:], xl_in)
                for gd in range(KTILES // 2):
                    emit_y_mm(gd)
                ag_y = finish_matvec(scale_inv8=False)

                # y post-AG: r0 = y blocks; transpose into V_it; gamma0
                # lazily; matvec-0 MMs interleaved per block.
                for j in range(NBLK):
                    nc.gpsimd.dma_start(blk(R_st, j), ag_y[64 * j : 64 * j + 64, :])
                    transpose_block(j, R_st)
                    for q in range(4 * j, 4 * j + 4):
                        emit_iter_mm(q)
                    scr = scr_pool.tile([128, SLICE], F32, name="g_scr", tag="scrx")
                    nc.vector.tensor_tensor(
                        out=half(scr, j), in0=blk(R_st, j), in1=blk(R_st, j),
                        op=ALU.mult,
                    )
                    nc.vector.reduce_sum(
                        part_col(gpart, j), half(scr, j), axis=AXX
                    )
                ag_w = finish_matvec(scale_inv8=True)

            # ================= CG iterations =================
            for k in range(K_ITERS):
                last = k == K_ITERS - 1
                # --- phase A: scatter w (bf16), delta partials, fold
                # t16 = w + lam*r so no w block outlives this loop ---
                for j in range(NBLK):
                    wb = scr_pool.tile([128, SLICE], BF16, name="wb", tag="scr16b")
                    nc.gpsimd.dma_start(
                        half(wb, j), ag_w[64 * j : 64 * j + 64, :]
                    )
                    scr = scr_pool.tile([128, SLICE], F32, name="d_scr", tag="scrx")
                    nc.vector.tensor_tensor(
                        out=half(scr, j), in0=half(wb, j), in1=blk(R_st, j),
                        op=ALU.mult,
                    )
                    nc.vector.reduce_sum(
                        part_col(dpart, j), half(scr, j), axis=AXX
                    )
                    nc.vector.scalar_tensor_tensor(
                        out=blk(T16, j), in0=blk(R_st, j), scalar=LAM,
                        in1=half(wb, j), op0=ALU.mult, op1=ALU.add,
                    )
                # --- scalars ---
                nc.vector.reduce_sum(g128[:], gpart[:], axis=AXX)
                nc.sync.dma_start(tmp64[:], g128[64:128, 0:1])
                nc.vector.tensor_tensor(
                    out=gamma[:], in0=tmp64[:], in1=g128[0:64, 0:1], op=ALU.add
                )
                nc.vector.reduce_sum(d128[:], dpart[:], axis=AXX)
                nc.sync.dma_start(tmp64[:], d128[64:128, 0:1])
                nc.vector.tensor_tensor(
                    out=d2[:], in0=tmp64[:], in1=d128[0:64, 0:1], op=ALU.add
                )
                nc.vector.scalar_tensor_tensor(
                    out=delta[:], in0=gamma[:], scalar=LAM, in1=d2[:],
                    op0=ALU.mult, op1=ALU.add,
                )
                if k == 0:
                    nc.vector.tensor_scalar_add(d2[:], delta[:], 1e-12)
                    nc.vector.reciprocal(inv_s[:], d2[:])
                    nc.vector.tensor_tensor(
                        out=alpha64[:], in0=gamma[:], in1=inv_s[:], op=ALU.mult
                    )
                else:
                    nc.vector.tensor_scalar_add(t1[:], g_old[:], 1e-12)
                    nc.vector.reciprocal(inv_s[:], t1[:])
                    nc.vector.tensor_tensor(
                        out=beta64[:], in0=gamma[:], in1=inv_s[:], op=ALU.mult
                    )
                    nc.vector.tensor_tensor(
                        out=t1[:], in0=gamma[:], in1=inv_a_old[:], op=ALU.mult
                    )
                    nc.vector.tensor_tensor(
                        out=t1[:], in0=t1[:], in1=beta64[:], op=ALU.mult
                    )
                    nc.vector.tensor_tensor(
                        out=d2[:], in0=delta[:], in1=t1[:], op=ALU.subtract
                    )
                    nc.vector.tensor_scalar_add(d2[:], d2[:], 1e-12)
                    nc.vector.reciprocal(inv_s[:], d2[:])
                    nc.vector.tensor_tensor(
                        out=alpha64[:], in0=gamma[:], in1=inv_s[:], op=ALU.mult
                    )
                    nc.vector.tensor_copy(beta128[0:64, 0:1], beta64[:])
                    nc.sync.dma_start(beta128[64:128, 0:1], beta128[0:64, 0:1])
                nc.vector.tensor_copy(g_old[:], gamma[:])
                nc.vector.reciprocal(inv_a_old[:], alpha64[:])
                nc.vector.tensor_copy(alpha128[0:64, 0:1], alpha64[:])
                nc.sync.dma_start(alpha128[64:128, 0:1], alpha128[0:64, 0:1])
                nc.vector.tensor_scalar_mul(nalpha128[:], alpha128[:], -1.0)

                # --- phase B per block: q, p, r updates; transposes; next MMs ---
                for j in range(NBLK):
                    rj = blk(R_st, j)
                    qj = blk(Q_st, j)
                    pj = blk(P_st, j)
                    if not last:
                        if k == 0:
                            nc.vector.tensor_copy(qj, blk(T16, j))
                            nc.vector.tensor_copy(pj, rj)
                        else:
                            nc.vector.scalar_tensor_tensor(
                                out=qj, in0=qj, scalar=sca(beta128, j),
                                in1=blk(T16, j), op0=ALU.mult, op1=ALU.add,
                            )
                            nc.vector.scalar_tensor_tensor(
                                out=pj, in0=pj, scalar=sca(beta128, j), in1=rj,
                                op0=ALU.mult, op1=ALU.add,
                            )
                        nc.vector.scalar_tensor_tensor(
                            out=rj, in0=qj, scalar=sca(nalpha128, j), in1=rj,
                            op0=ALU.mult, op1=ALU.add,
                        )
                        transpose_block(j, R_st)
                        for q in range(4 * j, 4 * j + 4):
                            emit_iter_mm(q)
                    else:
                        nc.vector.scalar_tensor_tensor(
                            out=pj, in0=pj, scalar=sca(beta128, j), in1=rj,
                            op0=ALU.mult, op1=ALU.add,
                        )

                # x updates (lazy; during next matvec) + gamma partials
                for j in range(NBLK):
                    pj = blk(P_st, j)
                    xw = scr_pool.tile([128, SLICE], F32, name="xw", tag="scrx")
                    if k == 0:
                        nc.vector.tensor_scalar_mul(
                            half(xw, j), pj, sca(alpha128, j)
                        )
                        nc.gpsimd.dma_start(blk(x_dram, j), half(xw, j))
                    else:
                        xr = scr_pool.tile([128, SLICE], F32, name="xr", tag="scrx")
                        nc.gpsimd.dma_start(half(xr, j), blk(x_dram, j))
                        nc.vector.scalar_tensor_tensor(
                            out=half(xw, j), in0=pj, scalar=sca(alpha128, j),
                            in1=half(xr, j), op0=ALU.mult, op1=ALU.add,
                        )
                        if last:
                            nc.gpsimd.dma_start(blk(z_out, j), half(xw, j))
                        else:
                            nc.gpsimd.dma_start(blk(x_dram, j), half(xw, j))
                    if not last:
                        rj = blk(R_st, j)
                        scr = scr_pool.tile([128, SLICE], F32, name="g_scr", tag="scrx")
                        nc.vector.tensor_tensor(
                            out=half(scr, j), in0=rj, in1=rj, op=ALU.mult,
                        )
                        nc.vector.reduce_sum(
                            part_col(gpart, j), half(scr, j), axis=AXX
                        )
                if not last:
                    ag_w = finish_matvec(scale_inv8=True)

    nc.compile()
    return nc


_NC_CACHE = None


def kernel(X_batch, rows, cols, values, num_users):
    global last_exec_time_ns, _NC_CACHE
    import ml_dtypes
    import scipy.sparse as sp

    X_batch = np.ascontiguousarray(np.asarray(X_batch, dtype=np.float32))
    rows = np.asarray(rows).astype(np.int64).ravel()
    cols = np.asarray(cols).astype(np.int64).ravel()
    values = np.asarray(values, dtype=np.float32).ravel()
    nu = int(np.asarray(num_users))

    Xs = sp.coo_matrix((values, (rows, cols)), shape=(nu, N_ITEMS)).tocsr()
    S = (Xs.T @ Xs).toarray().astype(np.float32, copy=False)
    s8_scale = np.float32(240.0 / max(np.abs(S).max(), 1e-9) / 1.05)
    inv8 = float(1.0 / s8_scale)
    S8 = np.clip(S * s8_scale, -240.0, 240.0).astype(ml_dtypes.float8_e4m3)
    S_hi = S.astype(ml_dtypes.bfloat16)

    xt = X_batch.T.astype(np.float32)                     # (items, batch)
    xt_t = np.ascontiguousarray(
        xt.reshape(KTILES, 128, BATCH).transpose(1, 0, 2).reshape(128, HALF)
    )
    xh = xt_t.astype(ml_dtypes.bfloat16)
    xl = (xt_t - xh.astype(np.float32)).astype(ml_dtypes.bfloat16)

    in_maps = []
    for c in range(N_CORES):
        sl = slice(c * SLICE, (c + 1) * SLICE)
        in_maps.append(
            {
                "s8": np.ascontiguousarray(S8[:, sl]),
                "shi": np.ascontiguousarray(S_hi[:, sl]),
                "xh": xh,
                "xl": xl,
            }
        )
    del S

    _install_ntff_hook()
    from concourse import bass_utils
    from concourse.bass_interp import get_hw_module

    if _NC_CACHE is None:
        nc = _build_bass(inv8)
        nc.m = get_hw_module(nc.m)
        _NC_CACHE = nc
    nc = _NC_CACHE

    try:
        res = bass_utils.run_bass_kernel_spmd(
            nc, in_maps, core_ids=list(range(N_CORES)), trace=True
        )
    except Exception:
        res = bass_utils.run_bass_kernel_spmd(
            nc, in_maps, core_ids=list(range(N_CORES)), trace=False
        )
    last_exec_time_ns = res.exec_time_ns

    z_st = res.results[0]["z_out"]                        # (128, HALF)
    Z = np.concatenate([z_st[0:64, :], z_st[64:128, :]], axis=1)  # (64, items)
    return Z.astype(np.float32)


# revision 22
# speedup vs baseline: 3.2140x; 1.0799x over previous
"""CG solve of (S + 500 I) Z = S X^T with S = X_coo^T X_coo, distributed
over 8 TRN2 NeuronCores.

Design (v3):
  - Host: S = X^T X dense (f32), shipped twice: bf16 (for the RHS pass
    y = S x, accuracy-critical) and fp8-e4m3 scaled (for the CG iteration
    matvecs, half the HBM traffic). Column-sharded 8 ways.
  - Matvec: out = lhsT.T @ rhs, lhsT = v items-major bf16, rhs = streamed
    S slab (fp8/bf16; mixed-dtype PE matmul HW-validated). Matmuls run in
    column-tiled pairs (tile_position (0,0)/(0,64)) so the M=64 batch
    fills the whole PE array: even/odd k-tiles (iters) or xh/xl (y pass)
    execute concurrently into psum partitions 0:64 / 64:128; the halves
    are folded with one PSUM->SBUF DMA + add.
  - Algorithm: Chronopoulos-Gear CG, one reduction point per iteration.
    gamma=(r,r) is computed lazily during the matvec; delta=(w,r) right
    after the AllGather (quarter-tile mult+reduce). Per-batch scalars are
    folded across partition halves with a ones-matmul on the otherwise
    idle PE (no cross-partition DMAs). Vector updates (q,r,p) are
    full-tile fused ops (p on GpSimd, concurrent with q/r on DVE);
    transposes then feed the next matvec block-by-block with its matmuls
    emitted interleaved. x accumulates in DRAM via GpSimd, off the
    critical path.
  - Iteration AllGathers carry bf16; y's carries f32.
  - 3 CG iterations; numpy mirror of exact device arithmetic: 6.7e-3
    max-rel vs the 2e-2 gate.
"""
import sys
import types

import numpy as np

N_CORES = 8
N_ITEMS = 16384
BATCH = 64
HALF = N_ITEMS // 2          # 8192
SLICE = N_ITEMS // N_CORES   # 2048
LAM = float(500.0)
K_ITERS = 3
KTILES = 128
NBLK = 8                     # rank blocks per gathered matvec

last_exec_time_ns = None


def _install_ntff_hook():
    if "antenv.axon_hooks" in sys.modules:
        return
    try:
        from trn_agent_boot.trn_boot import _ntff_profile_via_ctypes

        hook = _ntff_profile_via_ctypes("/opt/axon/libaxon_pjrt.so")
        mod = types.ModuleType("antenv.axon_hooks")
        mod.get_axon_ntff_profile_hook = lambda: hook
        mod.set_axon_ntff_profile_hook = lambda h: None
        sys.modules["antenv.axon_hooks"] = mod
    except Exception:
        pass


def _build_bass(inv8: float):
    import concourse.bass as bass  # noqa: F401
    import concourse.mybir as mybir
    import concourse.tile as tile
    from concourse import bacc
    from concourse.masks import make_identity

    F32 = mybir.dt.float32
    BF16 = mybir.dt.bfloat16
    FP8 = mybir.dt.float8e4
    ALU = mybir.AluOpType
    ACT_COPY = mybir.ActivationFunctionType.Copy
    AXX = mybir.AxisListType.X

    nc = bacc.Bacc(
        "TRN2",
        target_bir_lowering=False,
        debug=False,
        enable_asserts=False,
        num_devices=N_CORES,
    )

    s8_in = nc.dram_tensor("s8", [N_ITEMS, SLICE], FP8, kind="ExternalInput").ap()
    shi_in = nc.dram_tensor("shi", [N_ITEMS, SLICE], BF16, kind="ExternalInput").ap()
    xh_in = nc.dram_tensor("xh", [128, HALF], BF16, kind="ExternalInput").ap()
    xl_in = nc.dram_tensor("xl", [128, HALF], BF16, kind="ExternalInput").ap()
    z_out = nc.dram_tensor("z_out", [128, HALF], F32, kind="ExternalOutput").ap()

    s8_t = s8_in.rearrange("(g ki) m -> g ki m", ki=128)
    shi_t = shi_in.rearrange("(g ki) m -> g ki m", ki=128)

    with tile.TileContext(nc) as tc:
        with (
            tc.tile_pool(name="state", bufs=1) as state_pool,
            tc.tile_pool(name="scr", bufs=3) as scr_pool,
            tc.tile_pool(name="slab", bufs=4) as slab_pool,
            tc.tile_pool(name="sc", bufs=1) as sc_pool,
            tc.tile_pool(name="ps", bufs=1, space="PSUM") as ps_pool,
            tc.tile_pool(name="tps", bufs=2, space="PSUM") as tps_pool,
            tc.tile_pool(name="scps", bufs=1, space="PSUM") as scps_pool,
            tc.tile_pool(name="dram", bufs=2, space="DRAM") as dram_pool,
            tc.tile_pool(name="dramx", bufs=1, space="DRAM") as dramx_pool,
        ):
            R_st = state_pool.tile([128, HALF], F32, name="R_st")
            Q_st = state_pool.tile([128, HALF], BF16, name="Q_st")
            P_st = state_pool.tile([128, HALF], BF16, name="P_st")
            W16 = state_pool.tile([128, HALF], BF16, name="W16")
            V_it = state_pool.tile([128, HALF], BF16, name="V_it")
            x_dram = dramx_pool.tile([128, HALF], F32, name="x_dram")

            ident = sc_pool.tile([128, 128], F32, name="ident")
            make_identity(nc, ident[:])
            ident64 = sc_pool.tile([128, 64], F32, name="ident64")
            nc.vector.tensor_copy(ident64[0:64, :], ident[0:64, 0:64])
            nc.sync.dma_start(ident64[64:128, :], ident[0:64, 0:64])
            ones = sc_pool.tile([128, 128], F32, name="ones")
            nc.vector.memset(ones[:], 1.0)

            gpart = sc_pool.tile([128, 4], F32, name="gpart")
            dpart = sc_pool.tile([128, 4], F32, name="dpart")
            gp1 = sc_pool.tile([128, 1], F32, name="gp1")
            dp1 = sc_pool.tile([128, 1], F32, name="dp1")
            gamma = sc_pool.tile([128, 1], F32, name="gamma")
            g_old = sc_pool.tile([128, 1], F32, name="g_old")
            delta = sc_pool.tile([128, 1], F32, name="delta")
            t1 = sc_pool.tile([128, 1], F32, name="t1")
            d2 = sc_pool.tile([128, 1], F32, name="d2")
            inv_s = sc_pool.tile([128, 1], F32, name="inv_s")
            inv_a_old = sc_pool.tile([128, 1], F32, name="inv_a_old")
            alpha128 = sc_pool.tile([128, 1], F32, name="alpha128")
            nalpha128 = sc_pool.tile([128, 1], F32, name="nalpha128")
            beta128 = sc_pool.tile([128, 1], F32, name="beta128")

            def blk(tile_ap, j):
                h, qq = j // 4, j % 4
                return tile_ap[64 * h : 64 * h + 64, qq * SLICE : (qq + 1) * SLICE]

            def half(tile_ap, j):
                h = j // 4
                return tile_ap[64 * h : 64 * h + 64, :]

            def sca(vec128, j):
                h = j // 4
                return vec128[64 * h : 64 * h + 64, 0:1]

            mv_ps = [None]

            def emit_y_mm(gd):
                """y-pass matmuls for k-tile pair (2gd, 2gd+1); xh/xl
                col-tiled concurrently into psum halves."""
                slab = slab_pool.tile([128, 2 * SLICE], BF16, name="yslab", tag="slab")
                view = slab[:].rearrange("ki (u m) -> ki u m", u=2)
                nc.sync.dma_start(
                    view, shi_t[2 * gd : 2 * gd + 2].transpose([1, 0, 2])
                )
                ps = mv_ps[0]
                for u in range(2):
                    g = 2 * gd + u
                    for nt in range(SLICE // 512):
                        rh = slab[:, u * SLICE + nt * 512 : u * SLICE + (nt + 1) * 512]
                        nc.tensor.matmul(
                            ps[0:64, nt * 512 : (nt + 1) * 512],
                            lhsT=xh_ref[0][:, g * 64 : (g + 1) * 64], rhs=rh,
                            start=(g == 0), stop=(g == KTILES - 1),
                            tile_position=(0, 0), skip_group_check=True,
                        )
                        nc.tensor.matmul(
                            ps[64:128, nt * 512 : (nt + 1) * 512],
                            lhsT=xh_ref[1][:, g * 64 : (g + 1) * 64], rhs=rh,
                            start=(g == 0), stop=(g == KTILES - 1),
                            tile_position=(0, 64), skip_group_check=True,
                        )

            def emit_iter_mm(q):
                """iteration matvec matmuls for k-tile quad [4q, 4q+4);
                even/odd k-tiles col-tiled concurrently into psum halves."""
                slab = slab_pool.tile([128, 4 * SLICE], FP8, name="fslab", tag="slab")
                view = slab[:].rearrange("ki (u m) -> ki u m", u=4)
                nc.sync.dma_start(
                    view, s8_t[4 * q : 4 * q + 4].transpose([1, 0, 2])
                )
                ps = mv_ps[0]
                for up in range(2):
                    ge = 4 * q + 2 * up
                    go = ge + 1
                    for nt in range(SLICE // 512):
                        rh_e = slab[:, (2 * up) * SLICE + nt * 512
                                    : (2 * up) * SLICE + (nt + 1) * 512]
                        rh_o = slab[:, (2 * up + 1) * SLICE + nt * 512
                                    : (2 * up + 1) * SLICE + (nt + 1) * 512]
                        nc.tensor.matmul(
                            ps[0:64, nt * 512 : (nt + 1) * 512],
                            lhsT=V_it[:, ge * 64 : (ge + 1) * 64], rhs=rh_e,
                            start=(ge == 0), stop=(ge == KTILES - 2),
                            tile_position=(0, 0), skip_group_check=True,
                        )
                        nc.tensor.matmul(
                            ps[64:128, nt * 512 : (nt + 1) * 512],
                            lhsT=V_it[:, go * 64 : (go + 1) * 64], rhs=rh_o,
                            start=(go == 1), stop=(go == KTILES - 1),
                            tile_position=(0, 64), skip_group_check=True,
                        )

            def finish_matvec(scale_inv8):
                """fold psum halves (DMA + add) -> ag_in -> AllGather."""
                ps = mv_ps[0]
                fold = scr_pool.tile([128, SLICE], F32, name="fold", tag="scrx")
                nc.vector.tensor_copy(fold[64:128, :], ps[64:128, :])
                nc.sync.dma_start(fold[0:64, :], fold[64:128, :])
                nc.vector.tensor_tensor(
                    out=fold[0:64, :], in0=ps[0:64, :], in1=fold[0:64, :],
                    op=ALU.add,
                )
                if scale_inv8:
                    ag_in = dram_pool.tile(
                        [BATCH, SLICE], BF16, name="ag16_in", tag="ag16_in"
                    )
                    ag_out = dram_pool.tile(
                        [BATCH * N_CORES, SLICE], BF16, name="ag16_out",
                        addr_space="Shared", tag="ag16_out",
                    )
                    a_loc = scr_pool.tile(
                        [128, SLICE], BF16, name="a_loc16", tag="scr16b"
                    )
                    nc.vector.tensor_scalar_mul(
                        a_loc[0:64, :], fold[0:64, :], float(inv8)
                    )
                    nc.sync.dma_start(ag_in[:], a_loc[0:64, :])
                else:
                    ag_in = dram_pool.tile(
                        [BATCH, SLICE], F32, name="ag_in", tag="ag_in"
                    )
                    ag_out = dram_pool.tile(
                        [BATCH * N_CORES, SLICE], F32, name="ag_out",
                        addr_space="Shared", tag="ag_out",
                    )
                    nc.sync.dma_start(ag_in[:], fold[0:64, :])
                nc.gpsimd.collective_compute(
                    "AllGather",
                    ALU.bypass,
                    replica_groups=[list(range(N_CORES))],
                    ins=[ag_in[:].opt()],
                    outs=[ag_out[:].opt()],
                )
                return ag_out

            def scatter_all(ag_out, dst):
                """one DMA: (512, 2048) rank-major -> (128, 8192) state."""
                for h in range(2):
                    src = ag_out[256 * h : 256 * h + 256, :].rearrange(
                        "(rr b) m -> b rr m", rr=4, b=64
                    )
                    dsth = dst[64 * h : 64 * h + 64, :].rearrange(
                        "b (rr m) -> b rr m", rr=4
                    )
                    nc.gpsimd.dma_start(dsth, src)

            def transpose_block(j, src):
                """src block j (64, 2048) -> V_it items-major bf16."""
                h = j // 4
                cb = (j % 4) * SLICE
                for t8 in range(2):
                    tp = tps_pool.tile([128, 512], F32, name="tp")
                    for t in range(8):
                        tt = 8 * t8 + t
                        nc.tensor.transpose(
                            tp[:, t * 64 : (t + 1) * 64],
                            src[64 * h : 64 * h + 64, cb + 128 * tt : cb + 128 * (tt + 1)],
                            ident64[64 * h : 64 * h + 64, :],
                        )
                    c0 = (16 * j + 8 * t8) * 64
                    nc.scalar.activation(V_it[:, c0 : c0 + 512], tp[:], ACT_COPY)

            def dot_quarters(a, b, parts, p1):
                """per-batch dot partials of two (128, HALF) tiles -> p1
                (128,1), via 4 quarter mult+reduce on DVE."""
                for c in range(4):
                    sl = slice(c * SLICE, (c + 1) * SLICE)
                    scr = scr_pool.tile([128, SLICE], F32, name="dq", tag="scrx")
                    nc.vector.tensor_tensor(
                        out=scr[:], in0=a[:, sl], in1=b[:, sl], op=ALU.mult
                    )
                    nc.vector.reduce_sum(parts[:, c : c + 1], scr[:], axis=AXX)
                nc.vector.reduce_sum(p1[:], parts[:], axis=AXX)

            def halves_sum(p1, out128):
                """out128[m] = p1[0:64]+p1[64:128] for every m (ones-MM)."""
                pssc = scps_pool.tile([128, 1], F32, name="pssc", tag="pssc")
                nc.tensor.matmul(
                    pssc[:], lhsT=ones[:], rhs=p1[:],
                    start=True, stop=True, skip_group_check=True,
                )
                nc.vector.tensor_copy(out128[:], pssc[:])

            # ================= phase 0: load weights, y matvec =================
            mv_ps[0] = ps_pool.tile([128, SLICE], F32, name="mv_ps")
            xh_ref = [None, None]
            with tc.tile_pool(name="yw", bufs=1) as yw_pool:
                xh_ref[0] = yw_pool.tile([128, HALF], BF16, name="xh_t")
                xh_ref[1] = yw_pool.tile([128, HALF], BF16, name="xl_t")
                nc.sync.dma_start(xh_ref[0][:], xh_in)
                nc.sync.dma_start(xh_ref[1][:], xl_in)
                for gd in range(KTILES // 2):
                    emit_y_mm(gd)
                ag_y = finish_matvec(scale_inv8=False)

                # y post-AG: r0 = y (one scatter); transposes + matvec-0
                # MMs interleaved per block; gamma0 lazily during matvec-0.
                scatter_all(ag_y, R_st)
                for j in range(NBLK):
                    transpose_block(j, R_st)
                    for q in range(4 * j, 4 * j + 4):
                        emit_iter_mm(q)
                dot_quarters(R_st[:], R_st[:], gpart, gp1)
                ag_w = finish_matvec(scale_inv8=True)

            # ================= CG iterations =================
            for k in range(K_ITERS):
                last = k == K_ITERS - 1
                # --- phase A: scatter w (one DMA), delta partials ---
                scatter_all(ag_w, W16)
                dot_quarters(W16[:], R_st[:], dpart, dp1)
                # --- scalars on (128,1); PE ones-MM folds halves ---
                halves_sum(gp1, gamma)
                halves_sum(dp1, d2)
                nc.vector.scalar_tensor_tensor(
                    out=delta[:], in0=gamma[:], scalar=LAM, in1=d2[:],
                    op0=ALU.mult, op1=ALU.add,
                )
                if k == 0:
                    nc.vector.reciprocal(inv_s[:], delta[:])
                    nc.vector.tensor_tensor(
                        out=alpha128[:], in0=gamma[:], in1=inv_s[:], op=ALU.mult
                    )
                else:
                    nc.vector.reciprocal(inv_s[:], g_old[:])
                    nc.vector.tensor_tensor(
                        out=beta128[:], in0=gamma[:], in1=inv_s[:], op=ALU.mult
                    )
                    nc.vector.tensor_tensor(
                        out=t1[:], in0=gamma[:], in1=inv_a_old[:], op=ALU.mult
                    )
                    nc.vector.tensor_tensor(
                        out=t1[:], in0=t1[:], in1=beta128[:], op=ALU.mult
                    )
                    nc.vector.tensor_tensor(
                        out=d2[:], in0=delta[:], in1=t1[:], op=ALU.subtract
                    )
                    nc.vector.reciprocal(inv_s[:], d2[:])
                    nc.vector.tensor_tensor(
                        out=alpha128[:], in0=gamma[:], in1=inv_s[:], op=ALU.mult
                    )
                nc.vector.tensor_copy(g_old[:], gamma[:])
                nc.vector.reciprocal(inv_a_old[:], alpha128[:])
                nc.vector.tensor_scalar_mul(nalpha128[:], alpha128[:], -1.0)

                # --- vector updates: full-tile fused DVE ops. The p
                # recurrence uses p = beta*p + r_new + alpha*q_new (same
                # algebra as beta*p + r_old) so r updates in place first
                # and the transposes unblock as early as possible. ---
                if not last:
                    if k == 0:
                        nc.vector.scalar_tensor_tensor(
                            out=Q_st[:], in0=R_st[:], scalar=LAM, in1=W16[:],
                            op0=ALU.mult, op1=ALU.add,
                        )
                    else:
                        nc.vector.scalar_tensor_tensor(
                            out=Q_st[:], in0=Q_st[:], scalar=beta128[:],
                            in1=W16[:], op0=ALU.mult, op1=ALU.add,
                        )
                        nc.vector.scalar_tensor_tensor(
                            out=Q_st[:], in0=R_st[:], scalar=LAM, in1=Q_st[:],
                            op0=ALU.mult, op1=ALU.add,
                        )
                    nc.vector.scalar_tensor_tensor(
                        out=R_st[:], in0=Q_st[:], scalar=nalpha128[:],
                        in1=R_st[:], op0=ALU.mult, op1=ALU.add,
                    )
                    # transposes + next matvec MMs, block-pipelined
                    for j in range(NBLK):
                        transpose_block(j, R_st)
                        for q in range(4 * j, 4 * j + 4):
                            emit_iter_mm(q)
                    # lazy during the matvec: p update, gamma_{k+1}
                    if k == 0:
                        nc.vector.scalar_tensor_tensor(
                            out=P_st[:], in0=Q_st[:], scalar=alpha128[:],
                            in1=R_st[:], op0=ALU.mult, op1=ALU.add,
                        )
                    else:
                        nc.vector.scalar_tensor_tensor(
                            out=P_st[:], in0=P_st[:], scalar=beta128[:],
                            in1=R_st[:], op0=ALU.mult, op1=ALU.add,
                        )
                        nc.vector.scalar_tensor_tensor(
                            out=P_st[:], in0=Q_st[:], scalar=alpha128[:],
                            in1=P_st[:], op0=ALU.mult, op1=ALU.add,
                        )
                    dot_quarters(R_st[:], R_st[:], gpart, gp1)
                else:
                    nc.vector.scalar_tensor_tensor(
                        out=P_st[:], in0=P_st[:], scalar=beta128[:],
                        in1=R_st[:], op0=ALU.mult, op1=ALU.add,
                    )

                # x updates, lazy (DVE compute, gpsimd DMAs)
                for j in range(NBLK):
                    pj = blk(P_st, j)
                    xw = scr_pool.tile([128, SLICE], F32, name="xw", tag="scrx")
                    if k == 0:
                        nc.vector.tensor_scalar_mul(
                            half(xw, j), pj, sca(alpha128, j)
                        )
                        nc.gpsimd.dma_start(blk(x_dram, j), half(xw, j))
                    else:
                        xr = scr_pool.tile([128, SLICE], F32, name="xr", tag="scrx")
                        nc.gpsimd.dma_start(half(xr, j), blk(x_dram, j))
                        nc.vector.scalar_tensor_tensor(
                            out=half(xw, j), in0=pj, scalar=sca(alpha128, j),
                            in1=half(xr, j), op0=ALU.mult, op1=ALU.add,
                        )
                        dst = blk(z_out, j) if last else blk(x_dram, j)
                        nc.gpsimd.dma_start(dst, half(xw, j))
                if not last:
                    ag_w = finish_matvec(scale_inv8=True)

    nc.compile()
    return nc


_NC_CACHE = None


def kernel(X_batch, rows, cols, values, num_users):
    global last_exec_time_ns, _NC_CACHE
    import ml_dtypes
    import scipy.sparse as sp

    X_batch = np.ascontiguousarray(np.asarray(X_batch, dtype=np.float32))
    rows = np.asarray(rows).astype(np.int64).ravel()
    cols = np.asarray(cols).astype(np.int64).ravel()
    values = np.asarray(values, dtype=np.float32).ravel()
    nu = int(np.asarray(num_users))

    Xs = sp.coo_matrix((values, (rows, cols)), shape=(nu, N_ITEMS)).tocsr()
    S = (Xs.T @ Xs).toarray().astype(np.float32, copy=False)
    s8_scale = np.float32(240.0 / max(np.abs(S).max(), 1e-9) / 1.05)
    inv8 = float(1.0 / s8_scale)
    S8 = np.clip(S * s8_scale, -240.0, 240.0).astype(ml_dtypes.float8_e4m3)
    S_hi = S.astype(ml_dtypes.bfloat16)

    xt = X_batch.T.astype(np.float32)                     # (items, batch)
    xt_t = np.ascontiguousarray(
        xt.reshape(KTILES, 128, BATCH).transpose(1, 0, 2).reshape(128, HALF)
    )
    xh = xt_t.astype(ml_dtypes.bfloat16)
    xl = (xt_t - xh.astype(np.float32)).astype(ml_dtypes.bfloat16)

    in_maps = []
    for c in range(N_CORES):
        sl = slice(c * SLICE, (c + 1) * SLICE)
        in_maps.append(
            {
                "s8": np.ascontiguousarray(S8[:, sl]),
                "shi": np.ascontiguousarray(S_hi[:, sl]),
                "xh": xh,
                "xl": xl,
            }
        )
    del S

    _install_ntff_hook()
    from concourse import bass_utils
    from concourse.bass_interp import get_hw_module

    if _NC_CACHE is None:
        nc = _build_bass(inv8)
        nc.m = get_hw_module(nc.m)
        _NC_CACHE = nc
    nc = _NC_CACHE

    try:
        res = bass_utils.run_bass_kernel_spmd(
            nc, in_maps, core_ids=list(range(N_CORES)), trace=True
        )
    except Exception:
        res = bass_utils.run_bass_kernel_spmd(
            nc, in_maps, core_ids=list(range(N_CORES)), trace=False
        )
    last_exec_time_ns = res.exec_time_ns

    z_st = res.results[0]["z_out"]                        # (128, HALF)
    Z = np.concatenate([z_st[0:64, :], z_st[64:128, :]], axis=1)  # (64, items)
    return Z.astype(np.float32)


# revision 24
# speedup vs baseline: 3.2997x; 1.0267x over previous
"""CG solve of (S + 500 I) Z = S X^T with S = X_coo^T X_coo, distributed
over 8 TRN2 NeuronCores.

Design (v3):
  - Host: S = X^T X dense (f32), shipped twice: bf16 (for the RHS pass
    y = S x, accuracy-critical) and fp8-e4m3 scaled (for the CG iteration
    matvecs, half the HBM traffic). Column-sharded 8 ways.
  - Matvec: out = lhsT.T @ rhs, lhsT = v items-major bf16, rhs = streamed
    S slab (fp8/bf16; mixed-dtype PE matmul HW-validated). Matmuls run in
    column-tiled pairs (tile_position (0,0)/(0,64)) so the M=64 batch
    fills the whole PE array: even/odd k-tiles (iters) or xh/xl (y pass)
    execute concurrently into psum partitions 0:64 / 64:128; the halves
    are folded with one PSUM->SBUF DMA + add.
  - Algorithm: Chronopoulos-Gear CG, one reduction point per iteration.
    gamma=(r,r) is computed lazily during the matvec; delta=(w,r) right
    after the AllGather (quarter-tile mult+reduce). Per-batch scalars are
    folded across partition halves with a ones-matmul on the otherwise
    idle PE (no cross-partition DMAs). Vector updates (q,r,p) are
    full-tile fused ops (p on GpSimd, concurrent with q/r on DVE);
    transposes then feed the next matvec block-by-block with its matmuls
    emitted interleaved. x accumulates in DRAM via GpSimd, off the
    critical path.
  - Iteration AllGathers carry bf16; y's carries f32.
  - 3 CG iterations; numpy mirror of exact device arithmetic: 6.7e-3
    max-rel vs the 2e-2 gate.
"""
import sys
import types

import numpy as np

N_CORES = 8
N_ITEMS = 16384
BATCH = 64
HALF = N_ITEMS // 2          # 8192
SLICE = N_ITEMS // N_CORES   # 2048
LAM = float(500.0)
K_ITERS = 3
KTILES = 128
NBLK = 8                     # rank blocks per gathered matvec

last_exec_time_ns = None


def _install_ntff_hook():
    if "antenv.axon_hooks" in sys.modules:
        return
    try:
        from trn_agent_boot.trn_boot import _ntff_profile_via_ctypes

        hook = _ntff_profile_via_ctypes("/opt/axon/libaxon_pjrt.so")
        mod = types.ModuleType("antenv.axon_hooks")
        mod.get_axon_ntff_profile_hook = lambda: hook
        mod.set_axon_ntff_profile_hook = lambda h: None
        sys.modules["antenv.axon_hooks"] = mod
    except Exception:
        pass


def _build_bass(inv8: float):
    import concourse.bass as bass  # noqa: F401
    import concourse.mybir as mybir
    import concourse.tile as tile
    from concourse import bacc
    from concourse.masks import make_identity

    F32 = mybir.dt.float32
    BF16 = mybir.dt.bfloat16
    FP8 = mybir.dt.float8e4
    ALU = mybir.AluOpType
    ACT_COPY = mybir.ActivationFunctionType.Copy
    AXX = mybir.AxisListType.X

    nc = bacc.Bacc(
        "TRN2",
        target_bir_lowering=False,
        debug=False,
        enable_asserts=False,
        num_devices=N_CORES,
    )

    s8_in = nc.dram_tensor("s8", [N_ITEMS, SLICE], FP8, kind="ExternalInput").ap()
    shi_in = nc.dram_tensor("shi", [N_ITEMS, SLICE], BF16, kind="ExternalInput").ap()
    xh_in = nc.dram_tensor("xh", [128, HALF], BF16, kind="ExternalInput").ap()
    xl_in = nc.dram_tensor("xl", [128, HALF], BF16, kind="ExternalInput").ap()
    z_out = nc.dram_tensor("z_out", [128, HALF], F32, kind="ExternalOutput").ap()

    s8_t = s8_in.rearrange("(g ki) m -> g ki m", ki=128)
    shi_t = shi_in.rearrange("(g ki) m -> g ki m", ki=128)

    with tile.TileContext(nc) as tc:
        with (
            tc.tile_pool(name="state", bufs=1) as state_pool,
            tc.tile_pool(name="scr", bufs=3) as scr_pool,
            tc.tile_pool(name="slab", bufs=5) as slab_pool,
            tc.tile_pool(name="sc", bufs=1) as sc_pool,
            tc.tile_pool(name="ps", bufs=1, space="PSUM") as ps_pool,
            tc.tile_pool(name="tps", bufs=2, space="PSUM") as tps_pool,
            tc.tile_pool(name="scps", bufs=1, space="PSUM") as scps_pool,
            tc.tile_pool(name="dram", bufs=2, space="DRAM") as dram_pool,
            tc.tile_pool(name="dramx", bufs=1, space="DRAM") as dramx_pool,
        ):
            R_st = state_pool.tile([128, HALF], F32, name="R_st")
            Q_st = state_pool.tile([128, HALF], BF16, name="Q_st")
            P_st = state_pool.tile([128, HALF], BF16, name="P_st")
            W16 = state_pool.tile([128, HALF], BF16, name="W16")
            V_it = state_pool.tile([128, HALF], BF16, name="V_it")
            x_dram = dramx_pool.tile([128, HALF], F32, name="x_dram")

            ident = sc_pool.tile([128, 128], F32, name="ident")
            make_identity(nc, ident[:])
            ident64 = sc_pool.tile([128, 64], F32, name="ident64")
            nc.vector.tensor_copy(ident64[0:64, :], ident[0:64, 0:64])
            nc.sync.dma_start(ident64[64:128, :], ident[0:64, 0:64])
            ones = sc_pool.tile([128, 128], F32, name="ones")
            nc.vector.memset(ones[:], 1.0)

            gpart = sc_pool.tile([128, 4], F32, name="gpart")
            dpart = sc_pool.tile([128, 4], F32, name="dpart")
            gp1 = sc_pool.tile([128, 1], F32, name="gp1")
            dp1 = sc_pool.tile([128, 1], F32, name="dp1")
            gamma = sc_pool.tile([128, 1], F32, name="gamma")
            g_old = sc_pool.tile([128, 1], F32, name="g_old")
            delta = sc_pool.tile([128, 1], F32, name="delta")
            t1 = sc_pool.tile([128, 1], F32, name="t1")
            d2 = sc_pool.tile([128, 1], F32, name="d2")
            inv_s = sc_pool.tile([128, 1], F32, name="inv_s")
            inv_a_old = sc_pool.tile([128, 1], F32, name="inv_a_old")
            alpha128 = sc_pool.tile([128, 1], F32, name="alpha128")
            nalpha128 = sc_pool.tile([128, 1], F32, name="nalpha128")
            beta128 = sc_pool.tile([128, 1], F32, name="beta128")

            def blk(tile_ap, j):
                h, qq = j // 4, j % 4
                return tile_ap[64 * h : 64 * h + 64, qq * SLICE : (qq + 1) * SLICE]

            def half(tile_ap, j):
                h = j // 4
                return tile_ap[64 * h : 64 * h + 64, :]

            def sca(vec128, j):
                h = j // 4
                return vec128[64 * h : 64 * h + 64, 0:1]

            mv_ps = [None]

            def emit_y_mm(gd):
                """y-pass matmuls for k-tile pair (2gd, 2gd+1); xh/xl
                col-tiled concurrently into psum halves."""
                slab = slab_pool.tile([128, 2 * SLICE], BF16, name="yslab", tag="slab")
                view = slab[:].rearrange("ki (u m) -> ki u m", u=2)
                nc.sync.dma_start(
                    view, shi_t[2 * gd : 2 * gd + 2].transpose([1, 0, 2])
                )
                ps = mv_ps[0]
                for u in range(2):
                    g = 2 * gd + u
                    for nt in range(SLICE // 512):
                        rh = slab[:, u * SLICE + nt * 512 : u * SLICE + (nt + 1) * 512]
                        nc.tensor.matmul(
                            ps[0:64, nt * 512 : (nt + 1) * 512],
                            lhsT=xh_ref[0][:, g * 64 : (g + 1) * 64], rhs=rh,
                            start=(g == 0), stop=(g == KTILES - 1),
                            tile_position=(0, 0), skip_group_check=True,
                        )
                        nc.tensor.matmul(
                            ps[64:128, nt * 512 : (nt + 1) * 512],
                            lhsT=xh_ref[1][:, g * 64 : (g + 1) * 64], rhs=rh,
                            start=(g == 0), stop=(g == KTILES - 1),
                            tile_position=(0, 64), skip_group_check=True,
                        )

            def emit_iter_mm(q):
                """iteration matvec matmuls for k-tile quad [4q, 4q+4);
                even/odd k-tiles col-tiled concurrently into psum halves."""
                slab = slab_pool.tile([128, 4 * SLICE], FP8, name="fslab", tag="slab")
                view = slab[:].rearrange("ki (u m) -> ki u m", u=4)
                nc.sync.dma_start(
                    view, s8_t[4 * q : 4 * q + 4].transpose([1, 0, 2])
                )
                ps = mv_ps[0]
                for up in range(2):
                    ge = 4 * q + 2 * up
                    go = ge + 1
                    for nt in range(SLICE // 512):
                        rh_e = slab[:, (2 * up) * SLICE + nt * 512
                                    : (2 * up) * SLICE + (nt + 1) * 512]
                        rh_o = slab[:, (2 * up + 1) * SLICE + nt * 512
                                    : (2 * up + 1) * SLICE + (nt + 1) * 512]
                        nc.tensor.matmul(
                            ps[0:64, nt * 512 : (nt + 1) * 512],
                            lhsT=V_it[:, ge * 64 : (ge + 1) * 64], rhs=rh_e,
                            start=(ge == 0), stop=(ge == KTILES - 2),
                            tile_position=(0, 0), skip_group_check=True,
                        )
                        nc.tensor.matmul(
                            ps[64:128, nt * 512 : (nt + 1) * 512],
                            lhsT=V_it[:, go * 64 : (go + 1) * 64], rhs=rh_o,
                            start=(go == 1), stop=(go == KTILES - 1),
                            tile_position=(0, 64), skip_group_check=True,
                        )

            def finish_matvec(scale_inv8):
                """fold psum halves (DMA + add) -> ag_in -> AllGather."""
                ps = mv_ps[0]
                fold = scr_pool.tile([128, SLICE], F32, name="fold", tag="scrx")
                nc.vector.tensor_copy(fold[64:128, :], ps[64:128, :])
                nc.sync.dma_start(fold[0:64, :], fold[64:128, :])
                nc.vector.tensor_tensor(
                    out=fold[0:64, :], in0=ps[0:64, :], in1=fold[0:64, :],
                    op=ALU.add,
                )
                if scale_inv8:
                    ag_in = dram_pool.tile(
                        [BATCH, SLICE], BF16, name="ag16_in", tag="ag16_in"
                    )
                    ag_out = dram_pool.tile(
                        [BATCH * N_CORES, SLICE], BF16, name="ag16_out",
                        addr_space="Shared", tag="ag16_out",
                    )
                    a_loc = scr_pool.tile(
                        [128, SLICE], BF16, name="a_loc16", tag="scr16b"
                    )
                    nc.vector.tensor_scalar_mul(
                        a_loc[0:64, :], fold[0:64, :], float(inv8)
                    )
                    nc.sync.dma_start(ag_in[:], a_loc[0:64, :])
                else:
                    ag_in = dram_pool.tile(
                        [BATCH, SLICE], F32, name="ag_in", tag="ag_in"
                    )
                    ag_out = dram_pool.tile(
                        [BATCH * N_CORES, SLICE], F32, name="ag_out",
                        addr_space="Shared", tag="ag_out",
                    )
                    nc.sync.dma_start(ag_in[:], fold[0:64, :])
                nc.gpsimd.collective_compute(
                    "AllGather",
                    ALU.bypass,
                    replica_groups=[list(range(N_CORES))],
                    ins=[ag_in[:].opt()],
                    outs=[ag_out[:].opt()],
                )
                return ag_out

            def scatter_all(ag_out, dst):
                """one DMA: (512, 2048) rank-major -> (128, 8192) state."""
                for h in range(2):
                    src = ag_out[256 * h : 256 * h + 256, :].rearrange(
                        "(rr b) m -> b rr m", rr=4, b=64
                    )
                    dsth = dst[64 * h : 64 * h + 64, :].rearrange(
                        "b (rr m) -> b rr m", rr=4
                    )
                    nc.gpsimd.dma_start(dsth, src)

            def transpose_block(j, src):
                """src block j (64, 2048) -> V_it items-major bf16."""
                h = j // 4
                cb = (j % 4) * SLICE
                for t8 in range(2):
                    tp = tps_pool.tile([128, 512], F32, name="tp")
                    for t in range(8):
                        tt = 8 * t8 + t
                        nc.tensor.transpose(
                            tp[:, t * 64 : (t + 1) * 64],
                            src[64 * h : 64 * h + 64, cb + 128 * tt : cb + 128 * (tt + 1)],
                            ident64[64 * h : 64 * h + 64, :],
                        )
                    c0 = (16 * j + 8 * t8) * 64
                    nc.scalar.activation(V_it[:, c0 : c0 + 512], tp[:], ACT_COPY)

            def dot_quarters(a, b, parts, p1, split=False):
                """per-batch dot partials of two (128, HALF) tiles -> p1
                (128,1); quarters optionally split across DVE and Pool."""
                for c in range(4):
                    sl = slice(c * SLICE, (c + 1) * SLICE)
                    eng = nc.gpsimd if (split and c >= 2) else nc.vector
                    scr = scr_pool.tile([128, SLICE], F32, name="dq", tag="scrx")
                    eng.tensor_tensor(
                        out=scr[:], in0=a[:, sl], in1=b[:, sl], op=ALU.mult
                    )
                    nc.vector.reduce_sum(parts[:, c : c + 1], scr[:], axis=AXX)
                nc.vector.reduce_sum(p1[:], parts[:], axis=AXX)

            def halves_sum(p1, out128):
                """out128[m] = p1[0:64]+p1[64:128] for every m (ones-MM)."""
                pssc = scps_pool.tile([128, 1], F32, name="pssc", tag="pssc")
                nc.tensor.matmul(
                    pssc[:], lhsT=ones[:], rhs=p1[:],
                    start=True, stop=True, skip_group_check=True,
                )
                nc.vector.tensor_copy(out128[:], pssc[:])

            # ================= phase 0: load weights, y matvec =================
            mv_ps[0] = ps_pool.tile([128, SLICE], F32, name="mv_ps")
            xh_ref = [None, None]
            with tc.tile_pool(name="yw", bufs=1) as yw_pool:
                xh_ref[0] = yw_pool.tile([128, HALF], BF16, name="xh_t")
                xh_ref[1] = yw_pool.tile([128, HALF], BF16, name="xl_t")
                nc.sync.dma_start(xh_ref[0][:], xh_in)
                nc.sync.dma_start(xh_ref[1][:], xl_in)
                for gd in range(KTILES // 2):
                    emit_y_mm(gd)
                ag_y = finish_matvec(scale_inv8=False)

                # y post-AG: r0 = y (one scatter); transposes + matvec-0
                # MMs interleaved per block; gamma0 lazily during matvec-0.
                scatter_all(ag_y, R_st)
                for j in range(NBLK):
                    transpose_block(j, R_st)
                    for q in range(4 * j, 4 * j + 4):
                        emit_iter_mm(q)
                dot_quarters(R_st[:], R_st[:], gpart, gp1)
                ag_w = finish_matvec(scale_inv8=True)

            # ================= CG iterations =================
            for k in range(K_ITERS):
                last = k == K_ITERS - 1
                # --- phase A: scatter w (one DMA), delta partials ---
                scatter_all(ag_w, W16)
                dot_quarters(W16[:], R_st[:], dpart, dp1, split=True)
                # --- scalars on (128,1); PE ones-MM folds halves ---
                halves_sum(gp1, gamma)
                halves_sum(dp1, d2)
                nc.vector.scalar_tensor_tensor(
                    out=delta[:], in0=gamma[:], scalar=LAM, in1=d2[:],
                    op0=ALU.mult, op1=ALU.add,
                )
                if k == 0:
                    nc.vector.reciprocal(inv_s[:], delta[:])
                    nc.vector.tensor_tensor(
                        out=alpha128[:], in0=gamma[:], in1=inv_s[:], op=ALU.mult
                    )
                else:
                    nc.vector.reciprocal(inv_s[:], g_old[:])
                    nc.vector.tensor_tensor(
                        out=beta128[:], in0=gamma[:], in1=inv_s[:], op=ALU.mult
                    )
                    nc.vector.tensor_tensor(
                        out=t1[:], in0=gamma[:], in1=inv_a_old[:], op=ALU.mult
                    )
                    nc.vector.tensor_tensor(
                        out=t1[:], in0=t1[:], in1=beta128[:], op=ALU.mult
                    )
                    nc.vector.tensor_tensor(
                        out=d2[:], in0=delta[:], in1=t1[:], op=ALU.subtract
                    )
                    nc.vector.reciprocal(inv_s[:], d2[:])
                    nc.vector.tensor_tensor(
                        out=alpha128[:], in0=gamma[:], in1=inv_s[:], op=ALU.mult
                    )
                nc.vector.tensor_copy(g_old[:], gamma[:])
                nc.vector.reciprocal(inv_a_old[:], alpha128[:])
                nc.vector.tensor_scalar_mul(nalpha128[:], alpha128[:], -1.0)

                # --- vector updates: full-tile fused DVE ops. The p
                # recurrence uses p = beta*p + r_new + alpha*q_new (same
                # algebra as beta*p + r_old) so r updates in place first
                # and the transposes unblock as early as possible. ---
                if not last:
                    if k == 0:
                        nc.vector.scalar_tensor_tensor(
                            out=Q_st[:], in0=R_st[:], scalar=LAM, in1=W16[:],
                            op0=ALU.mult, op1=ALU.add,
                        )
                    else:
                        nc.vector.scalar_tensor_tensor(
                            out=Q_st[:], in0=Q_st[:], scalar=beta128[:],
                            in1=W16[:], op0=ALU.mult, op1=ALU.add,
                        )
                        nc.vector.scalar_tensor_tensor(
                            out=Q_st[:], in0=R_st[:], scalar=LAM, in1=Q_st[:],
                            op0=ALU.mult, op1=ALU.add,
                        )
                    nc.vector.scalar_tensor_tensor(
                        out=R_st[:], in0=Q_st[:], scalar=nalpha128[:],
                        in1=R_st[:], op0=ALU.mult, op1=ALU.add,
                    )
                    # transposes + next matvec MMs, block-pipelined
                    for j in range(NBLK):
                        transpose_block(j, R_st)
                        for q in range(4 * j, 4 * j + 4):
                            emit_iter_mm(q)
                    # lazy during the matvec: p update, gamma_{k+1}
                    if k == 0:
                        nc.vector.scalar_tensor_tensor(
                            out=P_st[:], in0=Q_st[:], scalar=alpha128[:],
                            in1=R_st[:], op0=ALU.mult, op1=ALU.add,
                        )
                    else:
                        nc.vector.scalar_tensor_tensor(
                            out=P_st[:], in0=P_st[:], scalar=beta128[:],
                            in1=R_st[:], op0=ALU.mult, op1=ALU.add,
                        )
                        nc.vector.scalar_tensor_tensor(
                            out=P_st[:], in0=Q_st[:], scalar=alpha128[:],
                            in1=P_st[:], op0=ALU.mult, op1=ALU.add,
                        )
                    dot_quarters(R_st[:], R_st[:], gpart, gp1)
                else:
                    nc.vector.scalar_tensor_tensor(
                        out=P_st[:], in0=P_st[:], scalar=beta128[:],
                        in1=R_st[:], op0=ALU.mult, op1=ALU.add,
                    )

                # x updates, lazy, quarter-tile (full 128 partitions)
                for qq in range(4):
                    csl = slice(qq * SLICE, (qq + 1) * SLICE)
                    xw = scr_pool.tile([128, SLICE], F32, name="xw", tag="scrx")
                    if k == 0:
                        nc.vector.tensor_scalar_mul(
                            xw[:], P_st[:, csl], alpha128[:]
                        )
                        nc.gpsimd.dma_start(x_dram[:, csl], xw[:])
                    else:
                        xr = scr_pool.tile([128, SLICE], F32, name="xr", tag="scrx")
                        nc.gpsimd.dma_start(xr[:], x_dram[:, csl])
                        nc.vector.scalar_tensor_tensor(
                            out=xw[:], in0=P_st[:, csl], scalar=alpha128[:],
                            in1=xr[:], op0=ALU.mult, op1=ALU.add,
                        )
                        dst = z_out[:, csl] if last else x_dram[:, csl]
                        nc.gpsimd.dma_start(dst, xw[:])
                if not last:
                    ag_w = finish_matvec(scale_inv8=True)

    nc.compile()
    return nc


_NC_CACHE = None


def kernel(X_batch, rows, cols, values, num_users):
    global last_exec_time_ns, _NC_CACHE
    import ml_dtypes
    import scipy.sparse as sp

    X_batch = np.ascontiguousarray(np.asarray(X_batch, dtype=np.float32))
    rows = np.asarray(rows).astype(np.int64).ravel()
    cols = np.asarray(cols).astype(np.int64).ravel()
    values = np.asarray(values, dtype=np.float32).ravel()
    nu = int(np.asarray(num_users))

    Xs = sp.coo_matrix((values, (rows, cols)), shape=(nu, N_ITEMS)).tocsr()
    S = (Xs.T @ Xs).toarray().astype(np.float32, copy=False)
    s8_scale = np.float32(240.0 / max(np.abs(S).max(), 1e-9) / 1.05)
    inv8 = float(1.0 / s8_scale)
    S8 = np.clip(S * s8_scale, -240.0, 240.0).astype(ml_dtypes.float8_e4m3)
    S_hi = S.astype(ml_dtypes.bfloat16)

    xt = X_batch.T.astype(np.float32)                     # (items, batch)
    xt_t = np.ascontiguousarray(
        xt.reshape(KTILES, 128, BATCH).transpose(1, 0, 2).reshape(128, HALF)
    )
    xh = xt_t.astype(ml_dtypes.bfloat16)
    xl = (xt_t - xh.astype(np.float32)).astype(ml_dtypes.bfloat16)

    in_maps = []
    for c in range(N_CORES):
        sl = slice(c * SLICE, (c + 1) * SLICE)
        in_maps.append(
            {
                "s8": np.ascontiguousarray(S8[:, sl]),
                "shi": np.ascontiguousarray(S_hi[:, sl]),
                "xh": xh,
                "xl": xl,
            }
        )
    del S

    _install_ntff_hook()
    from concourse import bass_utils
    from concourse.bass_interp import get_hw_module

    if _NC_CACHE is None:
        nc = _build_bass(inv8)
        nc.m = get_hw_module(nc.m)
        _NC_CACHE = nc
    nc = _NC_CACHE

    try:
        res = bass_utils.run_bass_kernel_spmd(
            nc, in_maps, core_ids=list(range(N_CORES)), trace=True
        )
    except Exception:
        res = bass_utils.run_bass_kernel_spmd(
            nc, in_maps, core_ids=list(range(N_CORES)), trace=False
        )
    last_exec_time_ns = res.exec_time_ns

    z_st = res.results[0]["z_out"]                        # (128, HALF)
    Z = np.concatenate([z_st[0:64, :], z_st[64:128, :]], axis=1)  # (64, items)
    return Z.astype(np.float32)


# revision 30
# speedup vs baseline: 3.7800x; 1.1456x over previous
"""CG solve of (S + 500 I) Z = S X^T with S = X_coo^T X_coo, distributed
over 8 TRN2 NeuronCores.

Design (v3):
  - Host: S = X^T X dense (f32), shipped twice: bf16 (for the RHS pass
    y = S x, accuracy-critical) and fp8-e4m3 scaled (for the CG iteration
    matvecs, half the HBM traffic). Column-sharded 8 ways.
  - Matvec: out = lhsT.T @ rhs, lhsT = v items-major bf16, rhs = streamed
    S slab (fp8/bf16; mixed-dtype PE matmul HW-validated). Matmuls run in
    column-tiled pairs (tile_position (0,0)/(0,64)) so the M=64 batch
    fills the whole PE array: even/odd k-tiles (iters) or xh/xl (y pass)
    execute concurrently into psum partitions 0:64 / 64:128; the halves
    are folded with one PSUM->SBUF DMA + add.
  - Algorithm: Chronopoulos-Gear CG, one reduction point per iteration.
    gamma=(r,r) is computed lazily during the matvec; delta=(w,r) right
    after the AllGather (quarter-tile mult+reduce). Per-batch scalars are
    folded across partition halves with a ones-matmul on the otherwise
    idle PE (no cross-partition DMAs). Vector updates (q,r,p) are
    full-tile fused ops (p on GpSimd, concurrent with q/r on DVE);
    transposes then feed the next matvec block-by-block with its matmuls
    emitted interleaved. x accumulates in DRAM via GpSimd, off the
    critical path.
  - Iteration AllGathers carry bf16; y's carries f32.
  - 3 CG iterations; numpy mirror of exact device arithmetic: 6.7e-3
    max-rel vs the 2e-2 gate.
"""
import sys
import types

import numpy as np

N_CORES = 8
N_ITEMS = 16384
BATCH = 64
HALF = N_ITEMS // 2          # 8192
SLICE = N_ITEMS // N_CORES   # 2048
LAM = float(500.0)
K_ITERS = 3
KTILES = 128
NBLK = 8                     # rank blocks per gathered matvec

last_exec_time_ns = None


def _install_ntff_hook():
    if "antenv.axon_hooks" in sys.modules:
        return
    try:
        from trn_agent_boot.trn_boot import _ntff_profile_via_ctypes

        hook = _ntff_profile_via_ctypes("/opt/axon/libaxon_pjrt.so")
        mod = types.ModuleType("antenv.axon_hooks")
        mod.get_axon_ntff_profile_hook = lambda: hook
        mod.set_axon_ntff_profile_hook = lambda h: None
        sys.modules["antenv.axon_hooks"] = mod
    except Exception:
        pass


def _build_bass(inv8: float, ncfac: float):
    import concourse.bass as bass  # noqa: F401
    import concourse.mybir as mybir
    import concourse.tile as tile
    from concourse import bacc
    from concourse.masks import make_identity

    F32 = mybir.dt.float32
    BF16 = mybir.dt.bfloat16
    FP8 = mybir.dt.float8e4
    ALU = mybir.AluOpType
    ACT_COPY = mybir.ActivationFunctionType.Copy
    AXX = mybir.AxisListType.X

    nc = bacc.Bacc(
        "TRN2",
        target_bir_lowering=False,
        debug=False,
        enable_asserts=False,
        num_devices=N_CORES,
    )

    s8_in = nc.dram_tensor("s8", [N_ITEMS, SLICE], FP8, kind="ExternalInput").ap()
    shi_in = nc.dram_tensor("shi", [N_ITEMS, SLICE], BF16, kind="ExternalInput").ap()
    xh_in = nc.dram_tensor("xh", [128, HALF], BF16, kind="ExternalInput").ap()
    xl_in = nc.dram_tensor("xl", [128, HALF], BF16, kind="ExternalInput").ap()
    u8_in = nc.dram_tensor("u8", [128, HALF], FP8, kind="ExternalInput").ap()
    z_out = nc.dram_tensor("z_out", [128, HALF], F32, kind="ExternalOutput").ap()

    s8_t = s8_in.rearrange("(g ki) m -> g ki m", ki=128)
    shi_t = shi_in.rearrange("(g ki) m -> g ki m", ki=128)

    with tile.TileContext(nc) as tc:
        with (
            tc.tile_pool(name="state", bufs=1) as state_pool,
            tc.tile_pool(name="scr", bufs=3) as scr_pool,
            tc.tile_pool(name="slab", bufs=4) as slab_pool,
            tc.tile_pool(name="sc", bufs=1) as sc_pool,
            tc.tile_pool(name="ps", bufs=1, space="PSUM") as ps_pool,
            tc.tile_pool(name="tps", bufs=2, space="PSUM") as tps_pool,
            tc.tile_pool(name="scps", bufs=1, space="PSUM") as scps_pool,
            tc.tile_pool(name="dram", bufs=2, space="DRAM") as dram_pool,
            tc.tile_pool(name="dramx", bufs=1, space="DRAM") as dramx_pool,
        ):
            R_st = state_pool.tile([128, HALF], F32, name="R_st")
            P_st = state_pool.tile([128, HALF], BF16, name="P_st")
            Z16 = state_pool.tile([128, HALF], BF16, name="Z16")
            W16 = state_pool.tile([128, HALF], BF16, name="W16")
            V_it = state_pool.tile([128, HALF], BF16, name="V_it")
            u8t = state_pool.tile([128, HALF], FP8, name="u8t")
            x_dram = dramx_pool.tile([128, HALF], F32, name="x_dram")

            ident = sc_pool.tile([128, 128], F32, name="ident")
            make_identity(nc, ident[:])
            ident64 = sc_pool.tile([128, 64], F32, name="ident64")
            nc.vector.tensor_copy(ident64[0:64, :], ident[0:64, 0:64])
            nc.sync.dma_start(ident64[64:128, :], ident[0:64, 0:64])
            ident64b = sc_pool.tile([128, 64], BF16, name="ident64b")
            nc.vector.tensor_copy(ident64b[:], ident64[:])
            foldm = sc_pool.tile([128, 128], F32, name="foldm")
            nc.vector.tensor_copy(foldm[:, 0:64], ident64[:])
            nc.vector.tensor_copy(foldm[:, 64:128], ident64[:])

            gpart = sc_pool.tile([128, 4], F32, name="gpart")
            zpart = sc_pool.tile([128, 4], F32, name="zpart")
            zp1 = sc_pool.tile([128, 1], F32, name="zp1")
            zeta = sc_pool.tile([128, 1], F32, name="zeta")
            cd128 = sc_pool.tile([128, 1], F32, name="cd128")
            nlal = sc_pool.tile([128, 1], F32, name="nlal")
            dpart = sc_pool.tile([128, 4], F32, name="dpart")
            gp1 = sc_pool.tile([128, 1], F32, name="gp1")
            dp1 = sc_pool.tile([128, 1], F32, name="dp1")
            gamma = sc_pool.tile([128, 1], F32, name="gamma")
            g_old = sc_pool.tile([128, 1], F32, name="g_old")
            delta = sc_pool.tile([128, 1], F32, name="delta")
            t1 = sc_pool.tile([128, 1], F32, name="t1")
            d2 = sc_pool.tile([128, 1], F32, name="d2")
            inv_s = sc_pool.tile([128, 1], F32, name="inv_s")
            inv_a_old = sc_pool.tile([128, 1], F32, name="inv_a_old")
            alpha128 = sc_pool.tile([128, 1], F32, name="alpha128")
            nalpha128 = sc_pool.tile([128, 1], F32, name="nalpha128")
            beta128 = sc_pool.tile([128, 1], F32, name="beta128")

            def blk(tile_ap, j):
                h, qq = j // 4, j % 4
                return tile_ap[64 * h : 64 * h + 64, qq * SLICE : (qq + 1) * SLICE]

            def half(tile_ap, j):
                h = j // 4
                return tile_ap[64 * h : 64 * h + 64, :]

            def sca(vec128, j):
                h = j // 4
                return vec128[64 * h : 64 * h + 64, 0:1]

            mv_ps = [None]

            def emit_y_mm(gd):
                """y-pass matmuls for k-tile pair (2gd, 2gd+1); xh/xl
                col-tiled concurrently into psum halves."""
                slab = slab_pool.tile([128, 2 * SLICE], BF16, name="yslab", tag="slab")
                view = slab[:].rearrange("ki (u m) -> ki u m", u=2)
                nc.sync.dma_start(
                    view, shi_t[2 * gd : 2 * gd + 2].transpose([1, 0, 2])
                )
                ps = mv_ps[0]
                for u in range(2):
                    g = 2 * gd + u
                    for nt in range(SLICE // 512):
                        rh = slab[:, u * SLICE + nt * 512 : u * SLICE + (nt + 1) * 512]
                        nc.tensor.matmul(
                            ps[0:64, nt * 512 : (nt + 1) * 512],
                            lhsT=xh_ref[0][:, g * 64 : (g + 1) * 64], rhs=rh,
                            start=(g == 0), stop=(g == KTILES - 1),
                            tile_position=(0, 0), skip_group_check=True,
                        )
                        nc.tensor.matmul(
                            ps[64:128, nt * 512 : (nt + 1) * 512],
                            lhsT=xh_ref[1][:, g * 64 : (g + 1) * 64], rhs=rh,
                            start=(g == 0), stop=(g == KTILES - 1),
                            tile_position=(0, 64), skip_group_check=True,
                        )

            def emit_iter_mm(q):
                """iteration matvec matmuls for k-tile quad [4q, 4q+4);
                even/odd k-tiles col-tiled concurrently into psum halves."""
                slab = slab_pool.tile([128, 4 * SLICE], FP8, name="fslab", tag="slab")
                view = slab[:].rearrange("ki (u m) -> ki u m", u=4)
                nc.sync.dma_start(
                    view, s8_t[4 * q : 4 * q + 4].transpose([1, 0, 2])
                )
                ps = mv_ps[0]
                for up in range(2):
                    ge = 4 * q + 2 * up
                    go = ge + 1
                    for nt in range(SLICE // 512):
                        rh_e = slab[:, (2 * up) * SLICE + nt * 512
                                    : (2 * up) * SLICE + (nt + 1) * 512]
                        rh_o = slab[:, (2 * up + 1) * SLICE + nt * 512
                                    : (2 * up + 1) * SLICE + (nt + 1) * 512]
                        nc.tensor.matmul(
                            ps[0:64, nt * 512 : (nt + 1) * 512],
                            lhsT=V_it[:, ge * 64 : (ge + 1) * 64], rhs=rh_e,
                            start=(ge == 0), stop=(ge == KTILES - 2),
                            tile_position=(0, 0), skip_group_check=True,
                        )
                        nc.tensor.matmul(
                            ps[64:128, nt * 512 : (nt + 1) * 512],
                            lhsT=V_it[:, go * 64 : (go + 1) * 64], rhs=rh_o,
                            start=(go == 1), stop=(go == KTILES - 1),
                            tile_position=(0, 64), skip_group_check=True,
                        )

            def finish_matvec(scale_inv8):
                """fold psum halves (DMA + add) -> ag_in -> AllGather."""
                ps = mv_ps[0]
                fold = scr_pool.tile([128, SLICE], F32, name="fold", tag="scrx")
                nc.vector.tensor_copy(fold[64:128, :], ps[64:128, :])
                nc.sync.dma_start(fold[0:64, :], fold[64:128, :])
                nc.vector.tensor_tensor(
                    out=fold[0:64, :], in0=ps[0:64, :], in1=fold[0:64, :],
                    op=ALU.add,
                )
                if scale_inv8:
                    ag_in = dram_pool.tile(
                        [BATCH, SLICE], BF16, name="ag16_in", tag="ag16_in"
                    )
                    ag_out = dram_pool.tile(
                        [BATCH * N_CORES, SLICE], BF16, name="ag16_out",
                        addr_space="Shared", tag="ag16_out",
                    )
                    a_loc = scr_pool.tile(
                        [128, SLICE], BF16, name="a_loc16", tag="scr16b"
                    )
                    nc.vector.tensor_scalar_mul(
                        a_loc[0:64, :], fold[0:64, :], float(inv8)
                    )
                    nc.sync.dma_start(ag_in[:], a_loc[0:64, :])
                else:
                    ag_in = dram_pool.tile(
                        [BATCH, SLICE], F32, name="ag_in", tag="ag_in"
                    )
                    ag_out = dram_pool.tile(
                        [BATCH * N_CORES, SLICE], F32, name="ag_out",
                        addr_space="Shared", tag="ag_out",
                    )
                    nc.sync.dma_start(ag_in[:], fold[0:64, :])
                nc.gpsimd.collective_compute(
                    "AllGather",
                    ALU.bypass,
                    replica_groups=[list(range(N_CORES))],
                    ins=[ag_in[:].opt()],
                    outs=[ag_out[:].opt()],
                )
                return ag_out

            def scatter_all(ag_out, dst):
                """one DMA: (512, 2048) rank-major -> (128, 8192) state."""
                for h in range(2):
                    src = ag_out[256 * h : 256 * h + 256, :].rearrange(
                        "(rr b) m -> b rr m", rr=4, b=64
                    )
                    dsth = dst[64 * h : 64 * h + 64, :].rearrange(
                        "b (rr m) -> b rr m", rr=4
                    )
                    nc.gpsimd.dma_start(dsth, src)

            def transpose_block(j, src):
                """src block j (64, 2048) -> V_it items-major bf16."""
                h = j // 4
                cb = (j % 4) * SLICE
                for t8 in range(2):
                    tp = tps_pool.tile([128, 512], BF16, name="tp")
                    for t in range(8):
                        tt = 8 * t8 + t
                        nc.tensor.transpose(
                            tp[:, t * 64 : (t + 1) * 64],
                            src[64 * h : 64 * h + 64, cb + 128 * tt : cb + 128 * (tt + 1)],
                            ident64b[64 * h : 64 * h + 64, :],
                        )
                    c0 = (16 * j + 8 * t8) * 64
                    nc.scalar.activation(V_it[:, c0 : c0 + 512], tp[:], ACT_COPY)

            def dot_quarters(a, b, parts, p1, split=False):
                """per-batch dot partials of two (128, HALF) tiles -> p1
                (128,1); quarters optionally split across DVE and Pool."""
                for c in range(4):
                    sl = slice(c * SLICE, (c + 1) * SLICE)
                    eng = nc.gpsimd if (split and c >= 2) else nc.vector
                    scr = scr_pool.tile([128, SLICE], F32, name="dq", tag="scrx")
                    eng.tensor_tensor(
                        out=scr[:], in0=a[:, sl], in1=b[:, sl], op=ALU.mult
                    )
                    nc.vector.reduce_sum(parts[:, c : c + 1], scr[:], axis=AXX)
                nc.vector.reduce_sum(p1[:], parts[:], axis=AXX)

            def halves_sum(p1, out128):
                """per-batch cross-half fold, broadcast to both halves:
                out128[m] = p1[m%64] + p1[64 + m%64] (fold-matrix MM)."""
                pssc = scps_pool.tile([128, 1], F32, name="pssc", tag="pssc")
                nc.tensor.matmul(
                    pssc[:], lhsT=foldm[:], rhs=p1[:],
                    start=True, stop=True, skip_group_check=True,
                )
                nc.vector.tensor_copy(out128[:], pssc[:])

            # ================= phase 0: load weights, y matvec =================
            mv_ps[0] = ps_pool.tile([128, SLICE], F32, name="mv_ps")
            xh_ref = [None, None]
            with tc.tile_pool(name="yw", bufs=1) as yw_pool:
                xh_ref[0] = yw_pool.tile([128, HALF], BF16, name="xh_t")
                xh_ref[1] = yw_pool.tile([128, HALF], BF16, name="xl_t")
                nc.sync.dma_start(xh_ref[0][:], xh_in)
                nc.sync.dma_start(xh_ref[1][:], xl_in)
                nc.gpsimd.dma_start(u8t[:], u8_in)
                for gd in range(KTILES // 2):
                    emit_y_mm(gd)
                ag_y = finish_matvec(scale_inv8=False)

                # y post-AG: r0 = y (one scatter); z0 = M^-1 r0 via the
                # rank-1 deflation (d = u.r per batch, z = r + ncfac*d*u);
                # transposes of z0 + matvec-0 MMs interleaved per block.
                scatter_all(ag_y, R_st)
                dot_quarters(u8t[:], R_st[:], dpart, dp1)
                halves_sum(dp1, d2)
                nc.vector.tensor_scalar_mul(cd128[:], d2[:], float(ncfac))
                nc.vector.scalar_tensor_tensor(
                    out=Z16[:], in0=u8t[:], scalar=cd128[:], in1=R_st[:],
                    op0=ALU.mult, op1=ALU.add,
                )
                for j in range(NBLK):
                    transpose_block(j, Z16)
                    for q in range(4 * j, 4 * j + 4):
                        emit_iter_mm(q)
                # lazy during matvec-0: gamma0=(r,z), zeta0=(z,z), p0=z0
                dot_quarters(R_st[:], Z16[:], gpart, gp1)
                dot_quarters(Z16[:], Z16[:], zpart, zp1)
                nc.vector.tensor_copy(P_st[:], Z16[:])
                ag_w = finish_matvec(scale_inv8=True)

            # ================= CG iterations (K=2, preconditioned) =========
            for k in range(2):
                last = k == 1
                # --- phase A: scatter w; delta = (w, z) + lam*zeta ---
                scatter_all(ag_w, W16)
                dot_quarters(W16[:], Z16[:], dpart, dp1, split=True)
                halves_sum(gp1, gamma)
                halves_sum(zp1, zeta)
                halves_sum(dp1, d2)
                nc.vector.scalar_tensor_tensor(
                    out=delta[:], in0=zeta[:], scalar=LAM, in1=d2[:],
                    op0=ALU.mult, op1=ALU.add,
                )
                if k == 0:
                    nc.vector.reciprocal(inv_s[:], delta[:])
                    nc.vector.tensor_tensor(
                        out=alpha128[:], in0=gamma[:], in1=inv_s[:], op=ALU.mult
                    )
                else:
                    nc.vector.reciprocal(inv_s[:], g_old[:])
                    nc.vector.tensor_tensor(
                        out=beta128[:], in0=gamma[:], in1=inv_s[:], op=ALU.mult
                    )
                    nc.vector.tensor_tensor(
                        out=t1[:], in0=gamma[:], in1=inv_a_old[:], op=ALU.mult
                    )
                    nc.vector.tensor_tensor(
                        out=t1[:], in0=t1[:], in1=beta128[:], op=ALU.mult
                    )
                    nc.vector.tensor_tensor(
                        out=d2[:], in0=delta[:], in1=t1[:], op=ALU.subtract
                    )
                    nc.vector.reciprocal(inv_s[:], d2[:])
                    nc.vector.tensor_tensor(
                        out=alpha128[:], in0=gamma[:], in1=inv_s[:], op=ALU.mult
                    )
                nc.vector.tensor_copy(g_old[:], gamma[:])
                nc.vector.reciprocal(inv_a_old[:], alpha128[:])
                nc.vector.tensor_scalar_mul(nalpha128[:], alpha128[:], -1.0)

                if not last:
                    # r1 = r0 - alpha*(w + lam*z): two fused STTs
                    nc.vector.tensor_scalar_mul(nlal[:], nalpha128[:], LAM)
                    nc.vector.scalar_tensor_tensor(
                        out=R_st[:], in0=Z16[:], scalar=nlal[:], in1=R_st[:],
                        op0=ALU.mult, op1=ALU.add,
                    )
                    nc.vector.scalar_tensor_tensor(
                        out=R_st[:], in0=W16[:], scalar=nalpha128[:],
                        in1=R_st[:], op0=ALU.mult, op1=ALU.add,
                    )
                    # z1 = r1 + ncfac*(u.r1)*u
                    dot_quarters(u8t[:], R_st[:], dpart, dp1)
                    halves_sum(dp1, d2)
                    nc.vector.tensor_scalar_mul(cd128[:], d2[:], float(ncfac))
                    nc.vector.scalar_tensor_tensor(
                        out=Z16[:], in0=u8t[:], scalar=cd128[:], in1=R_st[:],
                        op0=ALU.mult, op1=ALU.add,
                    )
                    # transposes + matvec-1 MMs, block-pipelined
                    for j in range(NBLK):
                        transpose_block(j, Z16)
                        for q in range(4 * j, 4 * j + 4):
                            emit_iter_mm(q)
                    # lazy during matvec-1: gamma1, zeta1
                    dot_quarters(R_st[:], Z16[:], gpart, gp1)
                    dot_quarters(Z16[:], Z16[:], zpart, zp1)
                else:
                    # p1 = z1 + beta*p0
                    nc.vector.scalar_tensor_tensor(
                        out=P_st[:], in0=P_st[:], scalar=beta128[:],
                        in1=Z16[:], op0=ALU.mult, op1=ALU.add,
                    )

                # x updates, lazy, quarter-tile (full 128 partitions)
                for qq in range(4):
                    csl = slice(qq * SLICE, (qq + 1) * SLICE)
                    xw = scr_pool.tile([128, SLICE], F32, name="xw", tag="scrx")
                    if k == 0:
                        nc.vector.tensor_scalar_mul(
                            xw[:], P_st[:, csl], alpha128[:]
                        )
                        nc.gpsimd.dma_start(x_dram[:, csl], xw[:])
                    else:
                        xr = scr_pool.tile([128, SLICE], F32, name="xr", tag="scrx")
                        nc.gpsimd.dma_start(xr[:], x_dram[:, csl])
                        nc.vector.scalar_tensor_tensor(
                            out=xw[:], in0=P_st[:, csl], scalar=alpha128[:],
                            in1=xr[:], op0=ALU.mult, op1=ALU.add,
                        )
                        nc.gpsimd.dma_start(z_out[:, csl], xw[:])
                if not last:
                    ag_w = finish_matvec(scale_inv8=True)

    nc.compile()
    return nc


_NC_CACHE = None


def kernel(X_batch, rows, cols, values, num_users):
    global last_exec_time_ns, _NC_CACHE
    import ml_dtypes
    import scipy.sparse as sp

    X_batch = np.ascontiguousarray(np.asarray(X_batch, dtype=np.float32))
    rows = np.asarray(rows).astype(np.int64).ravel()
    cols = np.asarray(cols).astype(np.int64).ravel()
    values = np.asarray(values, dtype=np.float32).ravel()
    nu = int(np.asarray(num_users))

    Xs = sp.coo_matrix((values, (rows, cols)), shape=(nu, N_ITEMS)).tocsr()
    S = (Xs.T @ Xs).toarray().astype(np.float32, copy=False)
    s8_scale = np.float32(240.0 / max(np.abs(S).max(), 1e-9) / 1.05)
    inv8 = float(1.0 / s8_scale)
    S8 = np.clip(S * s8_scale, -240.0, 240.0).astype(ml_dtypes.float8_e4m3)
    S_hi = S.astype(ml_dtypes.bfloat16)

    # rank-1 deflation preconditioner: dominant eigenvector of S via sparse
    # power iteration; M^-1 = I - c u u^T with c = 1 - (mu+lam)/(lmax+lam)
    u = np.random.default_rng(0).standard_normal(N_ITEMS).astype(np.float32)
    for _ in range(80):
        u = Xs.T @ (Xs @ u)
        u /= np.linalg.norm(u)
    lmax = float(u @ (Xs.T @ (Xs @ u)))
    mu = float(S.diagonal().mean())
    cdef = 1.0 - (mu + LAM) / (lmax + LAM)
    su = float(224.0 / max(np.abs(u).max(), 1e-30))
    ncfac = float(-cdef / (su * su))
    u_bc = np.vstack(
        [
            np.broadcast_to(u[:HALF] * su, (64, HALF)),
            np.broadcast_to(u[HALF:] * su, (64, HALF)),
        ]
    )
    u8 = np.clip(u_bc, -240.0, 240.0).astype(ml_dtypes.float8_e4m3)

    xt = X_batch.T.astype(np.float32)                     # (items, batch)
    xt_t = np.ascontiguousarray(
        xt.reshape(KTILES, 128, BATCH).transpose(1, 0, 2).reshape(128, HALF)
    )
    xh = xt_t.astype(ml_dtypes.bfloat16)
    xl = (xt_t - xh.astype(np.float32)).astype(ml_dtypes.bfloat16)

    in_maps = []
    for c in range(N_CORES):
        sl = slice(c * SLICE, (c + 1) * SLICE)
        in_maps.append(
            {
                "s8": np.ascontiguousarray(S8[:, sl]),
                "shi": np.ascontiguousarray(S_hi[:, sl]),
                "xh": xh,
                "xl": xl,
                "u8": u8,
            }
        )
    del S

    _install_ntff_hook()
    from concourse import bass_utils
    from concourse.bass_interp import get_hw_module

    if _NC_CACHE is None:
        nc = _build_bass(inv8, ncfac)
        nc.m = get_hw_module(nc.m)
        _NC_CACHE = nc
    nc = _NC_CACHE

    try:
        res = bass_utils.run_bass_kernel_spmd(
            nc, in_maps, core_ids=list(range(N_CORES)), trace=True
        )
    except Exception:
        res = bass_utils.run_bass_kernel_spmd(
            nc, in_maps, core_ids=list(range(N_CORES)), trace=False
        )
    last_exec_time_ns = res.exec_time_ns

    z_st = res.results[0]["z_out"]                        # (128, HALF)
    Z = np.concatenate([z_st[0:64, :], z_st[64:128, :]], axis=1)  # (64, items)
    return Z.astype(np.float32)


# revision 32
# speedup vs baseline: 4.1278x; 1.0920x over previous
"""CG solve of (S + 500 I) Z = S X^T with S = X_coo^T X_coo, distributed
over 8 TRN2 NeuronCores.

Design (v5): 855us HW (baseline 3237us), maxrel 3.6e-3 vs 2e-2 gate.
  - Host: S = X^T X dense (f32), shipped twice: bf16 (RHS pass y = S x,
    accuracy-critical, split-x weights) and fp8-e4m3 scaled (the two CG
    matvecs, half the HBM traffic). Column-sharded 8 ways. Host also runs
    a sparse power iteration for the dominant eigenvector u of S.
  - Preconditioned Chronopoulos-Gear CG, K=2: M^-1 = I - c u u^T deflates
    the single dominant (DC) eigenvalue (~412 vs the 500 I regularizer),
    after which the bulk spectrum is so tight that 2 iterations reach
    3.5e-3 (numpy bit-mirror of the device arithmetic).
  - Matvec: out = lhsT.T @ rhs, lhsT = z items-major bf16, rhs = streamed
    S slab (mixed-dtype bf16 x fp8 PE matmul, HW-validated). Matmuls run
    in column-tiled pairs (tile_position (0,0)/(0,64)) so the M=64 batch
    fills the full PE array; psum halves folded via copy + SBUF DMA + add.
  - One reduction point per iteration: gamma=(r,z), zeta=(z,z) computed
    lazily during the matvec; delta=(w,z) after the bf16 AllGather.
    Per-batch scalars folded across partition halves with a fold-matrix
    matmul on the otherwise-idle PE (no cross-partition DMAs). Vector
    updates are full-tile fused DVE ops; transposes feed the next matvec
    block-by-block with its matmuls emitted interleaved. x accumulates in
    DRAM off the critical path (quarter-tile updates).
"""
import sys
import types

import numpy as np

N_CORES = 8
N_ITEMS = 16384
BATCH = 64
HALF = N_ITEMS // 2          # 8192
SLICE = N_ITEMS // N_CORES   # 2048
LAM = float(500.0)
K_ITERS = 3
KTILES = 128
NBLK = 8                     # rank blocks per gathered matvec

last_exec_time_ns = None


def _install_ntff_hook():
    if "antenv.axon_hooks" in sys.modules:
        return
    try:
        from trn_agent_boot.trn_boot import _ntff_profile_via_ctypes

        hook = _ntff_profile_via_ctypes("/opt/axon/libaxon_pjrt.so")
        mod = types.ModuleType("antenv.axon_hooks")
        mod.get_axon_ntff_profile_hook = lambda: hook
        mod.set_axon_ntff_profile_hook = lambda h: None
        sys.modules["antenv.axon_hooks"] = mod
    except Exception:
        pass


def _build_bass(inv8: float, ncfac: float):
    import concourse.bass as bass  # noqa: F401
    import concourse.mybir as mybir
    import concourse.tile as tile
    from concourse import bacc
    from concourse.masks import make_identity

    F32 = mybir.dt.float32
    BF16 = mybir.dt.bfloat16
    FP8 = mybir.dt.float8e4
    ALU = mybir.AluOpType
    ACT_COPY = mybir.ActivationFunctionType.Copy
    AXX = mybir.AxisListType.X

    nc = bacc.Bacc(
        "TRN2",
        target_bir_lowering=False,
        debug=False,
        enable_asserts=False,
        num_devices=N_CORES,
    )

    s8_in = nc.dram_tensor("s8", [N_ITEMS, SLICE], FP8, kind="ExternalInput").ap()
    shi_in = nc.dram_tensor("shi", [N_ITEMS, SLICE], BF16, kind="ExternalInput").ap()
    xh_in = nc.dram_tensor("xh", [128, HALF], BF16, kind="ExternalInput").ap()
    xl_in = nc.dram_tensor("xl", [128, HALF], BF16, kind="ExternalInput").ap()
    u8_in = nc.dram_tensor("u8", [128, HALF], FP8, kind="ExternalInput").ap()
    z_out = nc.dram_tensor("z_out", [128, HALF], F32, kind="ExternalOutput").ap()

    s8_t = s8_in.rearrange("(g ki) m -> g ki m", ki=128)
    shi_t = shi_in.rearrange("(g ki) m -> g ki m", ki=128)

    with tile.TileContext(nc) as tc:
        with (
            tc.tile_pool(name="state", bufs=1) as state_pool,
            tc.tile_pool(name="scr", bufs=3) as scr_pool,
            tc.tile_pool(name="slab", bufs=4) as slab_pool,
            tc.tile_pool(name="sc", bufs=1) as sc_pool,
            tc.tile_pool(name="ps", bufs=1, space="PSUM") as ps_pool,
            tc.tile_pool(name="tps", bufs=2, space="PSUM") as tps_pool,
            tc.tile_pool(name="scps", bufs=1, space="PSUM") as scps_pool,
            tc.tile_pool(name="dram", bufs=2, space="DRAM") as dram_pool,
            tc.tile_pool(name="dramx", bufs=1, space="DRAM") as dramx_pool,
        ):
            R_st = state_pool.tile([128, HALF], F32, name="R_st")
            P_st = state_pool.tile([128, HALF], BF16, name="P_st")
            Z16 = state_pool.tile([128, HALF], BF16, name="Z16")
            W16 = state_pool.tile([128, HALF], BF16, name="W16")
            V_it = state_pool.tile([128, HALF], BF16, name="V_it")
            u8t = state_pool.tile([128, HALF], FP8, name="u8t")
            x_dram = dramx_pool.tile([128, HALF], F32, name="x_dram")

            ident = sc_pool.tile([128, 128], F32, name="ident")
            make_identity(nc, ident[:])
            ident64 = sc_pool.tile([128, 64], F32, name="ident64")
            nc.vector.tensor_copy(ident64[0:64, :], ident[0:64, 0:64])
            nc.sync.dma_start(ident64[64:128, :], ident[0:64, 0:64])
            ident64b = sc_pool.tile([128, 64], BF16, name="ident64b")
            nc.vector.tensor_copy(ident64b[:], ident64[:])
            foldm = sc_pool.tile([128, 128], F32, name="foldm")
            nc.vector.tensor_copy(foldm[:, 0:64], ident64[:])
            nc.vector.tensor_copy(foldm[:, 64:128], ident64[:])

            gpart = sc_pool.tile([128, 4], F32, name="gpart")
            zpart = sc_pool.tile([128, 4], F32, name="zpart")
            zp1 = sc_pool.tile([128, 1], F32, name="zp1")
            zeta = sc_pool.tile([128, 1], F32, name="zeta")
            cd128 = sc_pool.tile([128, 1], F32, name="cd128")
            nlal = sc_pool.tile([128, 1], F32, name="nlal")
            dpart = sc_pool.tile([128, 4], F32, name="dpart")
            gp1 = sc_pool.tile([128, 1], F32, name="gp1")
            dp1 = sc_pool.tile([128, 1], F32, name="dp1")
            gamma = sc_pool.tile([128, 1], F32, name="gamma")
            g_old = sc_pool.tile([128, 1], F32, name="g_old")
            delta = sc_pool.tile([128, 1], F32, name="delta")
            t1 = sc_pool.tile([128, 1], F32, name="t1")
            d2 = sc_pool.tile([128, 1], F32, name="d2")
            inv_s = sc_pool.tile([128, 1], F32, name="inv_s")
            inv_a_old = sc_pool.tile([128, 1], F32, name="inv_a_old")
            alpha128 = sc_pool.tile([128, 1], F32, name="alpha128")
            nalpha128 = sc_pool.tile([128, 1], F32, name="nalpha128")
            beta128 = sc_pool.tile([128, 1], F32, name="beta128")

            def blk(tile_ap, j):
                h, qq = j // 4, j % 4
                return tile_ap[64 * h : 64 * h + 64, qq * SLICE : (qq + 1) * SLICE]

            def half(tile_ap, j):
                h = j // 4
                return tile_ap[64 * h : 64 * h + 64, :]

            def sca(vec128, j):
                h = j // 4
                return vec128[64 * h : 64 * h + 64, 0:1]

            mv_ps = [None]

            def emit_y_mm(gd):
                """y-pass matmuls for k-tile pair (2gd, 2gd+1); xh/xl
                col-tiled concurrently into psum halves."""
                slab = slab_pool.tile([128, 2 * SLICE], BF16, name="yslab", tag="slab")
                view = slab[:].rearrange("ki (u m) -> ki u m", u=2)
                nc.sync.dma_start(
                    view, shi_t[2 * gd : 2 * gd + 2].transpose([1, 0, 2])
                )
                ps = mv_ps[0]
                for u in range(2):
                    g = 2 * gd + u
                    for nt in range(SLICE // 512):
                        rh = slab[:, u * SLICE + nt * 512 : u * SLICE + (nt + 1) * 512]
                        nc.tensor.matmul(
                            ps[0:64, nt * 512 : (nt + 1) * 512],
                            lhsT=xh_ref[0][:, g * 64 : (g + 1) * 64], rhs=rh,
                            start=(g == 0), stop=(g == KTILES - 1),
                            tile_position=(0, 0), skip_group_check=True,
                        )
                        nc.tensor.matmul(
                            ps[64:128, nt * 512 : (nt + 1) * 512],
                            lhsT=xh_ref[1][:, g * 64 : (g + 1) * 64], rhs=rh,
                            start=(g == 0), stop=(g == KTILES - 1),
                            tile_position=(0, 64), skip_group_check=True,
                        )

            def emit_iter_mm(q):
                """iteration matvec matmuls for k-tile quad [4q, 4q+4);
                even/odd k-tiles col-tiled concurrently into psum halves."""
                slab = slab_pool.tile([128, 4 * SLICE], FP8, name="fslab", tag="slab")
                view = slab[:].rearrange("ki (u m) -> ki u m", u=4)
                nc.sync.dma_start(
                    view, s8_t[4 * q : 4 * q + 4].transpose([1, 0, 2])
                )
                ps = mv_ps[0]
                for up in range(2):
                    ge = 4 * q + 2 * up
                    go = ge + 1
                    for nt in range(SLICE // 512):
                        rh_e = slab[:, (2 * up) * SLICE + nt * 512
                                    : (2 * up) * SLICE + (nt + 1) * 512]
                        rh_o = slab[:, (2 * up + 1) * SLICE + nt * 512
                                    : (2 * up + 1) * SLICE + (nt + 1) * 512]
                        nc.tensor.matmul(
                            ps[0:64, nt * 512 : (nt + 1) * 512],
                            lhsT=V_it[:, ge * 64 : (ge + 1) * 64], rhs=rh_e,
                            start=(ge == 0), stop=(ge == KTILES - 2),
                            tile_position=(0, 0), skip_group_check=True,
                        )
                        nc.tensor.matmul(
                            ps[64:128, nt * 512 : (nt + 1) * 512],
                            lhsT=V_it[:, go * 64 : (go + 1) * 64], rhs=rh_o,
                            start=(go == 1), stop=(go == KTILES - 1),
                            tile_position=(0, 64), skip_group_check=True,
                        )

            def finish_matvec(scale_inv8):
                """fold psum halves (DMA + add) -> ag_in -> AllGather."""
                ps = mv_ps[0]
                fold = scr_pool.tile([128, SLICE], F32, name="fold", tag="scrx")
                nc.vector.tensor_copy(fold[64:128, :], ps[64:128, :])
                nc.sync.dma_start(fold[0:64, :], fold[64:128, :])
                nc.vector.tensor_tensor(
                    out=fold[0:64, :], in0=ps[0:64, :], in1=fold[0:64, :],
                    op=ALU.add,
                )
                if scale_inv8:
                    ag_in = dram_pool.tile(
                        [BATCH, SLICE], BF16, name="ag16_in", tag="ag16_in"
                    )
                    ag_out = dram_pool.tile(
                        [BATCH * N_CORES, SLICE], BF16, name="ag16_out",
                        addr_space="Shared", tag="ag16_out",
                    )
                    a_loc = scr_pool.tile(
                        [128, SLICE], BF16, name="a_loc16", tag="scr16b"
                    )
                    nc.vector.tensor_scalar_mul(
                        a_loc[0:64, :], fold[0:64, :], float(inv8)
                    )
                    nc.sync.dma_start(ag_in[:], a_loc[0:64, :])
                else:
                    ag_in = dram_pool.tile(
                        [BATCH, SLICE], F32, name="ag_in", tag="ag_in"
                    )
                    ag_out = dram_pool.tile(
                        [BATCH * N_CORES, SLICE], F32, name="ag_out",
                        addr_space="Shared", tag="ag_out",
                    )
                    nc.sync.dma_start(ag_in[:], fold[0:64, :])
                nc.gpsimd.collective_compute(
                    "AllGather",
                    ALU.bypass,
                    replica_groups=[list(range(N_CORES))],
                    ins=[ag_in[:].opt()],
                    outs=[ag_out[:].opt()],
                )
                return ag_out

            def scatter_all(ag_out, dst):
                """one DMA: (512, 2048) rank-major -> (128, 8192) state."""
                for h in range(2):
                    src = ag_out[256 * h : 256 * h + 256, :].rearrange(
                        "(rr b) m -> b rr m", rr=4, b=64
                    )
                    dsth = dst[64 * h : 64 * h + 64, :].rearrange(
                        "b (rr m) -> b rr m", rr=4
                    )
                    nc.gpsimd.dma_start(dsth, src)

            def transpose_block(j, src):
                """src block j (64, 2048) -> V_it items-major bf16."""
                h = j // 4
                cb = (j % 4) * SLICE
                for t8 in range(2):
                    tp = tps_pool.tile([128, 512], BF16, name="tp")
                    for t in range(8):
                        tt = 8 * t8 + t
                        nc.tensor.transpose(
                            tp[:, t * 64 : (t + 1) * 64],
                            src[64 * h : 64 * h + 64, cb + 128 * tt : cb + 128 * (tt + 1)],
                            ident64b[64 * h : 64 * h + 64, :],
                        )
                    c0 = (16 * j + 8 * t8) * 64
                    nc.scalar.activation(V_it[:, c0 : c0 + 512], tp[:], ACT_COPY)

            def dot_quarters(a, b, parts, p1, split=False, pool_mult=False):
                """per-batch dot partials of two (128, HALF) tiles -> p1
                (128,1); mults optionally on the Pool engine."""
                for c in range(4):
                    sl = slice(c * SLICE, (c + 1) * SLICE)
                    eng = nc.gpsimd if (pool_mult or (split and c >= 2)) else nc.vector
                    scr = scr_pool.tile([128, SLICE], F32, name="dq", tag="scrx")
                    eng.tensor_tensor(
                        out=scr[:], in0=a[:, sl], in1=b[:, sl], op=ALU.mult
                    )
                    nc.vector.reduce_sum(parts[:, c : c + 1], scr[:], axis=AXX)
                nc.vector.reduce_sum(p1[:], parts[:], axis=AXX)

            def halves_sum(p1, out128):
                """per-batch cross-half fold, broadcast to both halves:
                out128[m] = p1[m%64] + p1[64 + m%64] (fold-matrix MM)."""
                pssc = scps_pool.tile([128, 1], F32, name="pssc", tag="pssc")
                nc.tensor.matmul(
                    pssc[:], lhsT=foldm[:], rhs=p1[:],
                    start=True, stop=True, skip_group_check=True,
                )
                nc.vector.tensor_copy(out128[:], pssc[:])

            # ================= phase 0: load weights, y matvec =================
            mv_ps[0] = ps_pool.tile([128, SLICE], F32, name="mv_ps")
            xh_ref = [None, None]
            with tc.tile_pool(name="yw", bufs=1) as yw_pool:
                xh_ref[0] = yw_pool.tile([128, HALF], BF16, name="xh_t")
                xh_ref[1] = yw_pool.tile([128, HALF], BF16, name="xl_t")
                nc.sync.dma_start(xh_ref[0][:], xh_in)
                nc.sync.dma_start(xh_ref[1][:], xl_in)
                nc.gpsimd.dma_start(u8t[:], u8_in)
                for gd in range(KTILES // 2):
                    emit_y_mm(gd)
                ag_y = finish_matvec(scale_inv8=False)

                # y post-AG: r0 = y (one scatter); z0 = M^-1 r0 via the
                # rank-1 deflation (d = u.r per batch, z = r + ncfac*d*u);
                # transposes of z0 + matvec-0 MMs interleaved per block.
                scatter_all(ag_y, R_st)
                dot_quarters(u8t[:], R_st[:], dpart, dp1)
                halves_sum(dp1, d2)
                nc.vector.tensor_scalar_mul(cd128[:], d2[:], float(ncfac))
                nc.vector.scalar_tensor_tensor(
                    out=Z16[:], in0=u8t[:], scalar=cd128[:], in1=R_st[:],
                    op0=ALU.mult, op1=ALU.add,
                )
                for j in range(NBLK):
                    transpose_block(j, Z16)
                    for q in range(4 * j, 4 * j + 4):
                        emit_iter_mm(q)
                ag_w = finish_matvec(scale_inv8=True)
                # lazy (during matvec-0 / AG): gamma0, zeta0, p0 = z0
                dot_quarters(R_st[:], Z16[:], gpart, gp1, pool_mult=True)
                dot_quarters(Z16[:], Z16[:], zpart, zp1, pool_mult=True)
                nc.vector.tensor_copy(P_st[:], Z16[:])

            # ================= CG iterations (K=2, preconditioned) =========
            xrp_ctx = tc.tile_pool(name="xrp", bufs=4)
            xrp_pool = xrp_ctx.__enter__()
            for k in range(2):
                last = k == 1
                # --- phase A: scatter w; delta = (w, z) + lam*zeta ---
                scatter_all(ag_w, W16)
                if last:
                    # prefetch the x quarters early (fire as soon as k=0's
                    # lazy x writes land, well inside matvec-1)
                    xrs = []
                    for qq in range(4):
                        xr = xrp_pool.tile([128, SLICE], F32, name="xr4")
                        nc.gpsimd.dma_start(
                            xr[:], x_dram[:, qq * SLICE : (qq + 1) * SLICE]
                        )
                        xrs.append(xr)
                dot_quarters(W16[:], Z16[:], dpart, dp1, split=True)
                halves_sum(gp1, gamma)
                halves_sum(zp1, zeta)
                halves_sum(dp1, d2)
                nc.vector.scalar_tensor_tensor(
                    out=delta[:], in0=zeta[:], scalar=LAM, in1=d2[:],
                    op0=ALU.mult, op1=ALU.add,
                )
                if k == 0:
                    nc.vector.reciprocal(inv_s[:], delta[:])
                    nc.vector.tensor_tensor(
                        out=alpha128[:], in0=gamma[:], in1=inv_s[:], op=ALU.mult
                    )
                else:
                    nc.vector.reciprocal(inv_s[:], g_old[:])
                    nc.vector.tensor_tensor(
                        out=beta128[:], in0=gamma[:], in1=inv_s[:], op=ALU.mult
                    )
                    nc.vector.tensor_tensor(
                        out=t1[:], in0=gamma[:], in1=inv_a_old[:], op=ALU.mult
                    )
                    nc.vector.tensor_tensor(
                        out=t1[:], in0=t1[:], in1=beta128[:], op=ALU.mult
                    )
                    nc.vector.tensor_tensor(
                        out=d2[:], in0=delta[:], in1=t1[:], op=ALU.subtract
                    )
                    nc.vector.reciprocal(inv_s[:], d2[:])
                    nc.vector.tensor_tensor(
                        out=alpha128[:], in0=gamma[:], in1=inv_s[:], op=ALU.mult
                    )
                nc.vector.tensor_copy(g_old[:], gamma[:])
                nc.vector.reciprocal(inv_a_old[:], alpha128[:])
                nc.vector.tensor_scalar_mul(nalpha128[:], alpha128[:], -1.0)

                if not last:
                    # r1 = r0 - alpha*(w + lam*z): two fused STTs
                    nc.vector.tensor_scalar_mul(nlal[:], nalpha128[:], LAM)
                    nc.vector.scalar_tensor_tensor(
                        out=R_st[:], in0=Z16[:], scalar=nlal[:], in1=R_st[:],
                        op0=ALU.mult, op1=ALU.add,
                    )
                    nc.vector.scalar_tensor_tensor(
                        out=R_st[:], in0=W16[:], scalar=nalpha128[:],
                        in1=R_st[:], op0=ALU.mult, op1=ALU.add,
                    )
                    # z1 = r1 + ncfac*(u.r1)*u
                    dot_quarters(u8t[:], R_st[:], dpart, dp1)
                    halves_sum(dp1, d2)
                    nc.vector.tensor_scalar_mul(cd128[:], d2[:], float(ncfac))
                    nc.vector.scalar_tensor_tensor(
                        out=Z16[:], in0=u8t[:], scalar=cd128[:], in1=R_st[:],
                        op0=ALU.mult, op1=ALU.add,
                    )
                    # transposes + matvec-1 MMs, block-pipelined
                    for j in range(NBLK):
                        transpose_block(j, Z16)
                        for q in range(4 * j, 4 * j + 4):
                            emit_iter_mm(q)
                    ag_w = finish_matvec(scale_inv8=True)
                    # lazy during matvec-1 / its AG: gamma1, zeta1
                    dot_quarters(R_st[:], Z16[:], gpart, gp1, pool_mult=True)
                    dot_quarters(Z16[:], Z16[:], zpart, zp1, pool_mult=True)
                else:
                    # p1 = z1 + beta*p0
                    nc.vector.scalar_tensor_tensor(
                        out=P_st[:], in0=P_st[:], scalar=beta128[:],
                        in1=Z16[:], op0=ALU.mult, op1=ALU.add,
                    )

                # x updates, lazy, quarter-tile (full 128 partitions)
                for qq in range(4):
                    csl = slice(qq * SLICE, (qq + 1) * SLICE)
                    xw = scr_pool.tile([128, SLICE], F32, name="xw", tag="scrx")
                    if k == 0:
                        nc.vector.tensor_scalar_mul(
                            xw[:], P_st[:, csl], alpha128[:]
                        )
                        nc.gpsimd.dma_start(x_dram[:, csl], xw[:])
                    else:
                        nc.vector.scalar_tensor_tensor(
                            out=xw[:], in0=P_st[:, csl], scalar=alpha128[:],
                            in1=xrs[qq][:], op0=ALU.mult, op1=ALU.add,
                        )
                        nc.gpsimd.dma_start(z_out[:, csl], xw[:])
            xrp_ctx.__exit__(None, None, None)

    nc.compile()
    return nc


_NC_CACHE = None


def kernel(X_batch, rows, cols, values, num_users):
    global last_exec_time_ns, _NC_CACHE
    import ml_dtypes
    import scipy.sparse as sp

    X_batch = np.ascontiguousarray(np.asarray(X_batch, dtype=np.float32))
    rows = np.asarray(rows).astype(np.int64).ravel()
    cols = np.asarray(cols).astype(np.int64).ravel()
    values = np.asarray(values, dtype=np.float32).ravel()
    nu = int(np.asarray(num_users))

    Xs = sp.coo_matrix((values, (rows, cols)), shape=(nu, N_ITEMS)).tocsr()
    S = (Xs.T @ Xs).toarray().astype(np.float32, copy=False)
    s8_scale = np.float32(240.0 / max(np.abs(S).max(), 1e-9) / 1.05)
    inv8 = float(1.0 / s8_scale)
    S8 = np.clip(S * s8_scale, -240.0, 240.0).astype(ml_dtypes.float8_e4m3)
    S_hi = S.astype(ml_dtypes.bfloat16)

    # rank-1 deflation preconditioner: dominant eigenvector of S via sparse
    # power iteration; M^-1 = I - c u u^T with c = 1 - (mu+lam)/(lmax+lam)
    u = np.random.default_rng(0).standard_normal(N_ITEMS).astype(np.float32)
    for _ in range(80):
        u = Xs.T @ (Xs @ u)
        u /= np.linalg.norm(u)
    lmax = float(u @ (Xs.T @ (Xs @ u)))
    mu = float(S.diagonal().mean())
    cdef = 1.0 - (mu + LAM) / (lmax + LAM)
    su = float(224.0 / max(np.abs(u).max(), 1e-30))
    ncfac = float(-cdef / (su * su))
    u_bc = np.vstack(
        [
            np.broadcast_to(u[:HALF] * su, (64, HALF)),
            np.broadcast_to(u[HALF:] * su, (64, HALF)),
        ]
    )
    u8 = np.clip(u_bc, -240.0, 240.0).astype(ml_dtypes.float8_e4m3)

    xt = X_batch.T.astype(np.float32)                     # (items, batch)
    xt_t = np.ascontiguousarray(
        xt.reshape(KTILES, 128, BATCH).transpose(1, 0, 2).reshape(128, HALF)
    )
    xh = xt_t.astype(ml_dtypes.bfloat16)
    xl = (xt_t - xh.astype(np.float32)).astype(ml_dtypes.bfloat16)

    in_maps = []
    for c in range(N_CORES):
        sl = slice(c * SLICE, (c + 1) * SLICE)
        in_maps.append(
            {
                "s8": np.ascontiguousarray(S8[:, sl]),
                "shi": np.ascontiguousarray(S_hi[:, sl]),
                "xh": xh,
                "xl": xl,
                "u8": u8,
            }
        )
    del S

    _install_ntff_hook()
    from concourse import bass_utils
    from concourse.bass_interp import get_hw_module

    if _NC_CACHE is None:
        nc = _build_bass(inv8, ncfac)
        nc.m = get_hw_module(nc.m)
        _NC_CACHE = nc
    nc = _NC_CACHE

    try:
        res = bass_utils.run_bass_kernel_spmd(
            nc, in_maps, core_ids=list(range(N_CORES)), trace=True
        )
    except Exception:
        res = bass_utils.run_bass_kernel_spmd(
            nc, in_maps, core_ids=list(range(N_CORES)), trace=False
        )
    last_exec_time_ns = res.exec_time_ns

    z_st = res.results[0]["z_out"]                        # (128, HALF)
    Z = np.concatenate([z_st[0:64, :], z_st[64:128, :]], axis=1)  # (64, items)
    return Z.astype(np.float32)
